# revision 1
# baseline (speedup 1.0000x reference)
"""EntityEncoder (gnn_message_passing) Trainium2 kernel — 8-core SPMD.

Strategy: edges are pre-partitioned on the host into 8 contiguous,
entity-aligned, edge-balanced shards (entity_indices is sorted, so each
entity's edges land wholly on one core — no cross-core collectives).
Within a core, segments are LPT-packed into 10 blocks of <=128 segments /
<=1280 edges; segment softmax + weighted segment-sums run as one-hot
matmuls on the tensor engine; count-table aggregation goes through an
A-matrix (segment x count) contracted against count_table; both output
projections run as bf16 matmuls against host-transposed weights.
"""
import sys
import numpy as np
import ml_dtypes

for _p in ("/root/.axon_site", "/root/.axon_site/_ro/trn_rl_repo",
           "/root/.axon_site/_ro/pypackages"):
    if _p not in sys.path:
        sys.path.append(_p)

import bass_rust
import concourse.bass as bass
import concourse.mybir as mybir
import concourse.tile as tile
from concourse.vector_clock import ScopedClock
from contextlib import ExitStack

BF16 = ml_dtypes.bfloat16
dt = mybir.dt
Alu = mybir.AluOpType
Act = mybir.ActivationFunctionType

# problem shape (hardcoded per contest contract)
N_CORES = 8
N = 100_000
P = 64
E = 10_000
D = 768
C = 1000
CPAD = 1024
OUT = 5120
# per-core packing
NBLK = 10
SPB = 128                # segs per block
CH = 10                  # chunks (of 128 edges) per block
EPB = CH * 128           # edges per block = 1280
NL = NBLK * EPB          # 12800 edge slots per core
E_PAD = NBLK * SPB       # 1280 seg slots per core
OH = OUT // 5            # 1024-wide output slab
PAD_SEG = 999.0


class _TileContextSplitDrain(tile.TileContext):
    """This container's walrus accepts only ONE sync wait per instruction
    ("Too many sync wait commands" in setupSyncWait). Split every extra wait
    onto a standalone same-engine NoOp placed immediately before the
    instruction — identical semantics, one wait per instruction."""

    def _lower_ordered_insts(self, ordered):
        for insts in ordered.values():
            if not any(
                i.sync_info is not None and len(i.sync_info.on_wait) > 1
                for i in insts
            ):
                continue
            new = []
            for inst in insts:
                si = inst.sync_info
                if si is not None and len(si.on_wait) > 1:
                    waits = list(si.on_wait)
                    for w in waits[:-1]:
                        nop = bass_rust.InstNoOp(
                            name=self.nc.get_next_instruction_name(),
                            ins=[], outs=[])
                        nop.engine = inst.engine
                        nop.sync_info = bass_rust.SyncInfo(
                            on_wait=[w], on_update=[])
                        new.append(nop)
                    si.on_wait = waits[-1:]
                new.append(inst)
            insts[:] = new
        return super()._lower_ordered_insts(ordered)

    def _drain_and_barrier(self, tick_clock, wait_clock):
        nc = self.nc
        drain_inst = nc.sync.drain()
        wait_clock.add_sem_waits(
            drain_inst.ins, ScopedClock({None: tick_clock.global_clock})
        )
        si = drain_inst.ins.sync_info
        if si is not None and len(si.on_wait) > 1:
            waits = list(si.on_wait)
            si.on_wait = waits[:1]
            for w in waits[1:]:
                n = nc.sync.nop()
                n.ins.sync_info = bass_rust.SyncInfo(on_wait=[w], on_update=[])
        nc.all_engine_barrier()
        assert self.sems is not None
        popped = nc._tile_sem_poison_stack.pop()
        assert popped is self._sem_poison
        nc.clear_and_free_semaphores(list(self.sems.allocated().values()))
        nc.all_engine_barrier()


# --------------------------------------------------------------------------
# host-side sharding / packing
# --------------------------------------------------------------------------

def _shard_and_pack(entity_indices):
    Nn = entity_indices.shape[0]
    starts = np.searchsorted(entity_indices, np.arange(E + 1))
    ideal = (np.arange(1, N_CORES) * Nn) // N_CORES
    ent_bnd = [0]
    for t in ideal:
        s = int(np.searchsorted(starts, t))
        if s > 0 and abs(int(starts[s - 1]) - int(t)) < abs(int(starts[s]) - int(t)):
            s -= 1
        ent_bnd.append(s)
    ent_bnd.append(E)

    cores = []
    for c in range(N_CORES):
        e_lo, e_hi = ent_bnd[c], ent_bnd[c + 1]
        segs = np.arange(e_lo, e_hi)
        sizes = (starts[e_lo + 1 : e_hi + 1] - starts[e_lo:e_hi]).astype(np.int64)
        n_edges = int(sizes.sum())
        assert e_hi - e_lo <= E_PAD and n_edges <= NL
        order = np.argsort(-sizes, kind="stable")
        blk_edges = [0] * NBLK
        blk_nseg = [0] * NBLK
        blk_segs = [[] for _ in range(NBLK)]
        for idx in order:
            sz = int(sizes[idx])
            best = -1
            for b in sorted(range(NBLK), key=lambda b: blk_edges[b]):
                if blk_nseg[b] < SPB and blk_edges[b] + sz <= EPB:
                    best = b
                    break
            assert best >= 0, "block packing overflow"
            blk_segs[best].append(int(segs[idx]))
            blk_edges[best] += sz
            blk_nseg[best] += 1
        perm = np.full(NL, -1, dtype=np.int64)
        seg_local = np.full(NL, PAD_SEG, dtype=np.float32)
        row2seg = np.full(E_PAD, -1, dtype=np.int64)
        inv_cnt = np.zeros(E_PAD, dtype=np.float32)
        for b in range(NBLK):
            pos = b * EPB
            for j, s in enumerate(blk_segs[b]):
                row = b * SPB + j
                row2seg[row] = s
                n = int(starts[s + 1] - starts[s])
                if n > 0:
                    inv_cnt[row] = 1.0 / n
                perm[pos : pos + n] = np.arange(starts[s], starts[s + 1])
                seg_local[pos : pos + n] = float(j)
                pos += n
        cores.append(dict(perm=perm, seg_local=seg_local, row2seg=row2seg,
                          inv_cnt=inv_cnt))
    return cores


# --------------------------------------------------------------------------
# device kernel
# --------------------------------------------------------------------------

def _build_nc():
    nc = bass.Bass("TRN2", target_bir_lowering=False, debug=False,
                   num_devices=N_CORES)

    f32, bf, f16, i32 = dt.float32, dt.bfloat16, dt.float16, dt.int32
    din = lambda n, s, d=f32: nc.dram_tensor(n, s, d, kind="ExternalInput")
    ent_d = din("ent", [NL, D])
    nbr_d = din("nbr", [NL, D])
    rel_d = din("rel", [NL, D])
    segl_d = din("segl", [NL])
    cnt_d = din("cntf", [NL])
    pr_d = din("prf", [NL])
    icnt_d = din("inv_cnt", [E_PAD])
    cscb_d = din("cscb", [128, CPAD], bf)
    pscb_d = din("pscb", [128, P], bf)
    wse_d = din("wse", [128, D], bf)
    wsn_d = din("wsn", [128, D], bf)
    wsr_d = din("wsr", [128, D], bf)
    ctp_d = din("ctp", [CPAD, D])
    wtr_d = din("wtr", [2 * D, OUT])
    wte_d = din("wte", [D, OUT])
    brel_d = din("brel", [OUT])
    bent_d = din("bent", [OUT])
    orel_d = nc.dram_tensor("orel", [E_PAD, OUT], f32, kind="ExternalOutput")
    oent_d = nc.dram_tensor("oent", [E_PAD, OUT], f32, kind="ExternalOutput")

    with _TileContextSplitDrain(nc) as tc, ExitStack() as es:
        const = es.enter_context(tc.tile_pool(name="const", bufs=1))
        accp = es.enter_context(tc.tile_pool(name="accp", bufs=1))

        # ---- constants ----
        iota_cnt = const.tile([128, CPAD], f16)
        iota_seg = const.tile([128, 128], bf)
        ident = const.tile([128, 128], bf)
        with tc.tile_pool(name="setup", bufs=1) as setup:
            iota_i = setup.tile([128, CPAD], i32)
            nc.gpsimd.iota(iota_i[:], pattern=[[1, CPAD]], base=0,
                           channel_multiplier=0)
            nc.vector.tensor_copy(iota_cnt[:], iota_i[:])
            nc.vector.tensor_copy(iota_seg[:], iota_i[:, 0:128])
            iota_ci = setup.tile([128, 1], i32)
            nc.gpsimd.iota(iota_ci[:], pattern=[[0, 1]], base=0,
                           channel_multiplier=1)
            iota_col = setup.tile([128, 1], f32)
            nc.vector.tensor_copy(iota_col[:], iota_ci[:])
            nc.vector.tensor_scalar(out=ident[:], in0=iota_seg[:],
                                    scalar1=iota_col[:],
                                    scalar2=None, op0=Alu.is_equal)
        ones_r = const.tile([1, 128], bf)
        nc.vector.memset(ones_r[:], 1.0)

        wse = const.tile([128, D], bf)
        nc.sync.dma_start(wse[:], wse_d.ap())
        wsn = const.tile([128, D], bf)
        nc.sync.dma_start(wsn[:], wsn_d.ap())
        wsr = const.tile([128, D], bf)
        nc.sync.dma_start(wsr[:], wsr_d.ap())
        cscb = const.tile([128, CPAD], bf)
        nc.sync.dma_start(cscb[:], cscb_d.ap())
        pscb = const.tile([128, P], bf)
        nc.sync.dma_start(pscb[:], pscb_d.ap())
        ctsb = const.tile([128, 8 * D], bf)
        nc.gpsimd.dma_start(
            ctsb[:], ctp_d.ap().rearrange("(i p) d -> p i d", p=128)
        )
        icnt_sb = const.tile([128, NBLK], f32)
        nc.sync.dma_start(
            icnt_sb[:], icnt_d.ap().rearrange("(b p) -> p b", p=128)
        )
        invd_sb = accp.tile([128, NBLK], f32)

        # resident transposed aggregates, one tile per (feat-chunk, block) so
        # projection reads depend only on their own block's writes
        relcatT = [[accp.tile([128, 128], bf, name=f"relcatT{t}_{b}",
                              tag=f"relcatT{t}_{b}") for b in range(NBLK)]
                   for t in range(12)]
        entT = [[accp.tile([128, 128], bf, name=f"entT{t}_{b}",
                           tag=f"entT{t}_{b}") for b in range(NBLK)]
                for t in range(6)]

        # ---- merged aggregation + projection (Tile interleaves by deps) ----
        HD = CH // 2 * D  # half-block embedding width (5 chunks)
        with tc.tile_pool(name="edges", bufs=2) as edges, \
             tc.tile_pool(name="chunkp", bufs=2) as chunkp, \
             tc.tile_pool(name="evac", bufs=2) as evac, \
             tc.tile_pool(name="wpool", bufs=2) as wpool, \
             tc.tile_pool(name="outp", bufs=2) as outp, \
             tc.tile_pool(name="psagg", bufs=1, space="PSUM") as psagg, \
             tc.tile_pool(name="pp", bufs=2, space="PSUM") as pp:
            for b in range(NBLK):
                halves = []
                for hb in range(2):
                    r0 = b * EPB + hb * (EPB // 2)
                    r1 = r0 + EPB // 2
                    enth = edges.tile([128, HD], bf, tag="enth")
                    nc.gpsimd.dma_start(
                        enth[:],
                        ent_d.ap()[r0:r1, :].rearrange("(p j) d -> p j d", j=CH // 2),
                    )
                    nbrh = edges.tile([128, HD], bf, tag="nbrh")
                    nc.gpsimd.dma_start(
                        nbrh[:],
                        nbr_d.ap()[r0:r1, :].rearrange("(p j) d -> p j d", j=CH // 2),
                    )
                    relh = edges.tile([128, HD], bf, tag="relh")
                    nc.gpsimd.dma_start(
                        relh[:],
                        rel_d.ap()[r0:r1, :].rearrange("(p j) d -> p j d", j=CH // 2),
                    )
                    slh = edges.tile([128, CH // 2], f32, tag="slh")
                    nc.sync.dma_start(
                        slh[:], segl_d.ap()[r0:r1].rearrange("(p j) -> p j", j=CH // 2))
                    cnh = edges.tile([128, CH // 2], f32, tag="cnh")
                    nc.sync.dma_start(
                        cnh[:], cnt_d.ap()[r0:r1].rearrange("(p j) -> p j", j=CH // 2))
                    prh = edges.tile([128, CH // 2], f32, tag="prh")
                    nc.sync.dma_start(
                        prh[:], pr_d.ap()[r0:r1].rearrange("(p j) -> p j", j=CH // 2))
                    halves.append((enth, nbrh, relh, slh, cnh, prh))

                ps_rel = psagg.tile([128, D], f32, tag="ps_rel")
                ps_A = psagg.tile([128, CPAD], f32, tag="ps_A")
                ps_ent = psagg.tile([128, D], f32, tag="ps_ent")

                for j in range(CH):
                    enth, nbrh, relh, slh, cnh, prh = halves[j // 5]
                    jj = j % 5
                    ej = enth[:, jj * D : (jj + 1) * D]
                    nj = nbrh[:, jj * D : (jj + 1) * D]
                    rj = relh[:, jj * D : (jj + 1) * D]
                    scratch = chunkp.tile([128, CPAD], bf, tag="scratch")
                    scr = scratch[:, 0:D]
                    sa = chunkp.tile([128, 1], f32, tag="sa")
                    nc.vector.scalar_tensor_tensor(
                        out=scr, in0=ej, scalar=1.0, in1=wse[:],
                        op0=Alu.mult, op1=Alu.mult, accum_out=sa[:])
                    sb_ = chunkp.tile([128, 1], f32, tag="sb_")
                    nc.vector.scalar_tensor_tensor(
                        out=scr, in0=nj, scalar=1.0, in1=wsn[:],
                        op0=Alu.mult, op1=Alu.mult, accum_out=sb_[:])
                    sc_ = chunkp.tile([128, 1], f32, tag="sc_")
                    nc.vector.scalar_tensor_tensor(
                        out=scr, in0=rj, scalar=1.0, in1=wsr[:],
                        op0=Alu.mult, op1=Alu.mult, accum_out=sc_[:])
                    oc = chunkp.tile([128, CPAD], bf, tag="oc")
                    nc.vector.tensor_scalar(out=oc[:], in0=iota_cnt[:],
                                            scalar1=cnh[:, jj : jj + 1],
                                            scalar2=None, op0=Alu.is_equal)
                    nc.vector.memset(oc[:, CPAD - 1 : CPAD], 1.0)
                    sd_ = chunkp.tile([128, 1], f32, tag="sd_")
                    nc.vector.scalar_tensor_tensor(
                        out=scratch[:], in0=oc[:], scalar=1.0, in1=cscb[:],
                        op0=Alu.mult, op1=Alu.mult, accum_out=sd_[:])
                    op_ = chunkp.tile([128, P], bf, tag="op_")
                    nc.vector.tensor_scalar(out=op_[:], in0=iota_cnt[:, 0:P],
                                            scalar1=prh[:, jj : jj + 1],
                                            scalar2=None, op0=Alu.is_equal)
                    se_ = chunkp.tile([128, 1], f32, tag="se_")
                    nc.vector.scalar_tensor_tensor(
                        out=scratch[:, 0:P], in0=op_[:], scalar=1.0, in1=pscb[:],
                        op0=Alu.mult, op1=Alu.mult, accum_out=se_[:])
                    t1_ = chunkp.tile([128, 1], f32, tag="t1_")
                    nc.vector.tensor_scalar(out=t1_[:], in0=sa[:], scalar1=sb_[:],
                                            scalar2=sc_[:], op0=Alu.add, op1=Alu.add)
                    t2_ = chunkp.tile([128, 1], f32, tag="t2_")
                    nc.vector.tensor_scalar(out=t2_[:], in0=sd_[:], scalar1=se_[:],
                                            scalar2=None, op0=Alu.add)
                    ex_ = chunkp.tile([128, 1], f32, tag="ex_")
                    nc.scalar.activation(ex_[:], t1_[:], Act.Exp, bias=t2_[:])
                    oh = chunkp.tile([128, 128], bf, tag="oh")
                    nc.vector.tensor_scalar(out=oh[:], in0=iota_seg[:],
                                            scalar1=slh[:, jj : jj + 1],
                                            scalar2=None, op0=Alu.is_equal)
                    ohx = chunkp.tile([128, 128], bf, tag="ohx")
                    nc.vector.tensor_scalar(out=ohx[:], in0=iota_seg[:],
                                            scalar1=slh[:, jj : jj + 1],
                                            scalar2=ex_[:],
                                            op0=Alu.is_equal, op1=Alu.mult)
                    st, sp = (j == 0), (j == CH - 1)
                    nc.tensor.matmul(ps_rel[:, 0:512], ohx[:], rj[:, 0:512],
                                     start=st, stop=sp)
                    nc.tensor.matmul(ps_rel[:, 512:D], ohx[:], rj[:, 512:D],
                                     start=st, stop=sp)
                    nc.tensor.matmul(ps_A[:, 0:512], ohx[:], oc[:, 0:512],
                                     start=st, stop=sp)
                    nc.tensor.matmul(ps_A[:, 512:CPAD], ohx[:], oc[:, 512:CPAD],
                                     start=st, stop=sp)
                    nc.tensor.matmul(ps_ent[:, 0:512], oh[:], ej[:, 0:512],
                                     start=st, stop=sp)
                    nc.tensor.matmul(ps_ent[:, 512:D], oh[:], ej[:, 512:D],
                                     start=st, stop=sp)

                # block epilogue
                dmx = chunkp.tile([128, 1], f32, tag="dmx")
                nc.vector.tensor_scalar(out=dmx[:], in0=ps_A[:, CPAD - 1 : CPAD],
                                        scalar1=1e-30, scalar2=None, op0=Alu.max)
                nc.vector.reciprocal(invd_sb[:, b : b + 1], dmx[:])
                relsb = evac.tile([128, D], bf, tag="relsb")
                nc.scalar.activation(relsb[:], ps_rel[:], Act.Copy,
                                     scale=invd_sb[:, b : b + 1])
                Asb = evac.tile([128, CPAD], bf, tag="Asb")
                nc.scalar.activation(Asb[:], ps_A[:], Act.Copy,
                                     scale=invd_sb[:, b : b + 1])
                entsb = evac.tile([128, D], bf, tag="entsb")
                nc.scalar.activation(entsb[:], ps_ent[:], Act.Copy,
                                     scale=icnt_sb[:, b : b + 1])

                bs = slice(b * 128, (b + 1) * 128)
                for t in range(6):
                    pt = pp.tile([128, 512], bf, tag="pp")
                    nc.tensor.transpose(pt[:, 0:128], relsb[:, t * 128 : (t + 1) * 128],
                                        ident[:])
                    nc.scalar.activation(relcatT[t][b][:], pt[:, 0:128], Act.Copy)
                    pt2 = pp.tile([128, 512], bf, tag="pp")
                    nc.tensor.transpose(pt2[:, 0:128], entsb[:, t * 128 : (t + 1) * 128],
                                        ident[:])
                    nc.scalar.activation(entT[t][b][:], pt2[:, 0:128], Act.Copy)
                ATl = []
                for t in range(8):
                    pt3 = pp.tile([128, 512], bf, tag="pp")
                    nc.tensor.transpose(pt3[:, 0:128], Asb[:, t * 128 : (t + 1) * 128],
                                        ident[:])
                    at = evac.tile([128, 128], bf, name=f"AT{t}", tag=f"AT{t}")
                    nc.scalar.activation(at[:], pt3[:, 0:128], Act.Copy)
                    ATl.append(at)
                for dchunk in range(6):
                    pc = pp.tile([128, 512], f32, tag="pp")
                    for cc in range(8):
                        nc.tensor.matmul(
                            pc[:, 0:128],
                            ctsb[:, cc * D + dchunk * 128 : cc * D + (dchunk + 1) * 128],
                            ATl[cc][:],
                            start=(cc == 0), stop=(cc == 7))
                    nc.scalar.activation(relcatT[6 + dchunk][b][:], pc[:, 0:128],
                                         Act.Copy)

            # ---- projections (interleave with later aggregation blocks) ----
            for (Tt, wt_d, b_d, o_d, KC) in (
                (relcatT, wtr_d, brel_d, orel_d, 12),
                (entT, wte_d, bent_d, oent_d, 6),
            ):
                for h in range(5):
                    wt = wpool.tile([128, KC * OH], bf, tag="wt")
                    for k in range(KC):
                        nc.gpsimd.dma_start(
                            wt[:, k * OH : (k + 1) * OH],
                            wt_d.ap()[k * 128 : (k + 1) * 128,
                                      h * OH : (h + 1) * OH],
                        )
                    bt = wpool.tile([1, OH], bf, tag="bt")
                    nc.gpsimd.dma_start(
                        bt[:],
                        b_d.ap()[h * OH : (h + 1) * OH].rearrange(
                            "(o c) -> o c", o=1),
                    )
                    for sblk in range(NBLK):
                        stage = outp.tile([128, OH], f32, tag="stage")
                        for oc5 in range(OH // 512):
                            pso = pp.tile([128, 512], f32, tag="pp")
                            nc.tensor.matmul(pso[:], ones_r[:],
                                             bt[:, oc5 * 512 : (oc5 + 1) * 512],
                                             start=True, stop=False)
                            for k in range(KC):
                                nc.tensor.matmul(
                                    pso[:],
                                    Tt[k][sblk][:],
                                    wt[:, k * OH + oc5 * 512 : k * OH + (oc5 + 1) * 512],
                                    start=False, stop=(k == KC - 1))
                            if oc5 % 2 == 0:
                                nc.vector.tensor_copy(
                                    stage[:, oc5 * 512 : (oc5 + 1) * 512], pso[:])
                            else:
                                nc.scalar.activation(
                                    stage[:, oc5 * 512 : (oc5 + 1) * 512], pso[:],
                                    Act.Copy)
                        nc.sync.dma_start(
                            o_d.ap()[sblk * 128 : (sblk + 1) * 128,
                                     h * OH : (h + 1) * OH],
                            stage[:],
                        )
    return nc


_NC_CACHE = None


def _get_nc():
    global _NC_CACHE
    if _NC_CACHE is None:
        _NC_CACHE = _build_nc()
    return _NC_CACHE


# --------------------------------------------------------------------------
# entry point
# --------------------------------------------------------------------------

def kernel(prompt_embs, entity_embs, neighbor_embs, relation_embs,
           count_table, scorer_W, scorer_b, rel_W, rel_b, ent_W, ent_b,
           counts, prompt_indices, entity_indices):
    from concourse.bass_utils import run_bass_kernel_spmd

    prompt_embs = np.asarray(prompt_embs, dtype=np.float32)
    entity_embs = np.asarray(entity_embs, dtype=np.float32)
    neighbor_embs = np.asarray(neighbor_embs, dtype=np.float32)
    relation_embs = np.asarray(relation_embs, dtype=np.float32)
    count_table = np.asarray(count_table, dtype=np.float32)
    scorer_W = np.asarray(scorer_W, dtype=np.float32)
    scorer_b = np.asarray(scorer_b, dtype=np.float32)
    rel_W = np.asarray(rel_W, dtype=np.float32)
    rel_b = np.asarray(rel_b, dtype=np.float32)
    ent_W = np.asarray(ent_W, dtype=np.float32)
    ent_b = np.asarray(ent_b, dtype=np.float32)
    counts = np.asarray(counts)
    prompt_indices = np.asarray(prompt_indices)
    entity_indices = np.asarray(entity_indices)

    cores = _shard_and_pack(entity_indices)

    # replicated (weight-derived) host prep
    w = scorer_W[0]
    w1, w2, w3, w4, w5 = (w[i * D : (i + 1) * D] for i in range(5))
    pscore = (prompt_embs * w1[None, :]).sum(1) + scorer_b[0]     # fold bias
    cscore = (count_table * w5[None, :]).sum(1)
    cs_pad = np.zeros(CPAD, np.float32)
    cs_pad[:C] = cscore
    cscb = np.broadcast_to(cs_pad.astype(BF16), (128, CPAD)).copy()
    pscb = np.broadcast_to(pscore.astype(BF16), (128, P)).copy()
    wse = np.broadcast_to(w2.astype(BF16), (128, D)).copy()
    wsn = np.broadcast_to(w3.astype(BF16), (128, D)).copy()
    wsr = np.broadcast_to(w4.astype(BF16), (128, D)).copy()
    ctp = np.zeros((CPAD, D), np.float32)
    ctp[:C] = count_table
    wtr = np.ascontiguousarray(rel_W.T)     # [2D, OUT]
    wte = np.ascontiguousarray(ent_W.T)     # [D, OUT]

    in_maps = []
    for core in cores:
        perm = core["perm"]
        valid = perm >= 0
        src = np.where(valid, perm, 0)

        def take2d(a):
            out = a[src]
            out[~valid] = 0.0
            return np.ascontiguousarray(out)

        def take1d(a):
            out = a.astype(np.float32)[src]
            out[~valid] = 0.0
            return np.ascontiguousarray(out)

        in_maps.append(dict(
            ent=take2d(entity_embs), nbr=take2d(neighbor_embs),
            rel=take2d(relation_embs),
            segl=core["seg_local"], cntf=take1d(counts),
            prf=take1d(prompt_indices), inv_cnt=core["inv_cnt"],
            cscb=cscb, pscb=pscb, wse=wse, wsn=wsn, wsr=wsr,
            ctp=ctp, wtr=wtr, wte=wte, brel=rel_b, bent=ent_b,
        ))

    nc = _get_nc()
    res = run_bass_kernel_spmd(nc, in_maps, list(range(N_CORES)))

    rel_out = np.zeros((E, OUT), np.float32)
    ent_out = np.zeros((E, OUT), np.float32)
    for c, core in enumerate(cores):
        rows = core["row2seg"]
        mask = rows >= 0
        rel_out[rows[mask]] = res.results[c]["orel"][mask]
        ent_out[rows[mask]] = res.results[c]["oent"][mask]
    return rel_out, ent_out



# revision 6
# speedup vs baseline: 1.3236x; 1.3236x over previous
"""EntityEncoder (gnn_message_passing) Trainium2 kernel — 8-core SPMD.

Strategy: edges are pre-partitioned on the host into 8 contiguous,
entity-aligned, edge-balanced shards (entity_indices is sorted, so each
entity's edges land wholly on one core — no cross-core collectives).
Within a core, segments are LPT-packed into 10 blocks of <=128 segments /
<=1280 edges.  All HBM traffic is bf16.  The host folds the prompt-score,
count-score and scorer bias into a per-edge prescore, gathers per-edge
count embeddings (with an appended ones column that yields the softmax
denominator for free), and pre-tiles the projection weights.  On device:
one fused 2304-col dot per 128-edge chunk (vector), exp on scalar,
one-hot segment matmuls on tensor for the three segment reductions,
PE transposes of the [seg,feat] aggregates, then bf16 output projections.
Projection bias and the final row scatter are applied on the host.
"""
import sys
import numpy as np
import ml_dtypes

for _p in ("/root/.axon_site", "/root/.axon_site/_ro/trn_rl_repo",
           "/root/.axon_site/_ro/pypackages"):
    if _p not in sys.path:
        sys.path.append(_p)

import bass_rust
import concourse.bass as bass
import concourse.mybir as mybir
import concourse.tile as tile
from concourse.vector_clock import ScopedClock
from contextlib import ExitStack

BF16 = ml_dtypes.bfloat16
dt = mybir.dt
Alu = mybir.AluOpType
Act = mybir.ActivationFunctionType

# problem shape (hardcoded per contest contract)
N_CORES = 8
N = 100_000
P = 64
E = 10_000
D = 768
C = 1000
OUT = 5120
# per-core packing
NBLK = 10
SPB = 128                # segs per block
CH = 10                  # chunks (of 128 edges) per block
EPB = CH * 128           # edges per block = 1280
NL = NBLK * EPB          # 12800 edge slots per core
E_PAD = NBLK * SPB       # 1280 seg slots per core
OH = OUT // 5            # 1024-wide output slab
PAD_SEG = 999.0
CW = 776                 # count-emb cols: 768 + ones col + 7 pad
ED = 3 * D               # merged [ent|nbr|rel] width = 2304


class _TileContextSplitDrain(tile.TileContext):
    """This container's walrus accepts only ONE sync wait per instruction
    ("Too many sync wait commands" in setupSyncWait). Split every extra wait
    onto a standalone same-engine NoOp placed immediately before the
    instruction — identical semantics, one wait per instruction."""

    def _lower_ordered_insts(self, ordered):
        for insts in ordered.values():
            if not any(
                i.sync_info is not None and len(i.sync_info.on_wait) > 1
                for i in insts
            ):
                continue
            new = []
            for inst in insts:
                si = inst.sync_info
                if si is not None and len(si.on_wait) > 1:
                    waits = list(si.on_wait)
                    for w in waits[:-1]:
                        nop = bass_rust.InstNoOp(
                            name=self.nc.get_next_instruction_name(),
                            ins=[], outs=[])
                        nop.engine = inst.engine
                        nop.sync_info = bass_rust.SyncInfo(
                            on_wait=[w], on_update=[])
                        new.append(nop)
                    si.on_wait = waits[-1:]
                new.append(inst)
            insts[:] = new
        return super()._lower_ordered_insts(ordered)

    def _drain_and_barrier(self, tick_clock, wait_clock):
        nc = self.nc
        drain_inst = nc.sync.drain()
        wait_clock.add_sem_waits(
            drain_inst.ins, ScopedClock({None: tick_clock.global_clock})
        )
        si = drain_inst.ins.sync_info
        if si is not None and len(si.on_wait) > 1:
            waits = list(si.on_wait)
            si.on_wait = waits[:1]
            for w in waits[1:]:
                n = nc.sync.nop()
                n.ins.sync_info = bass_rust.SyncInfo(on_wait=[w], on_update=[])
        nc.all_engine_barrier()
        assert self.sems is not None
        popped = nc._tile_sem_poison_stack.pop()
        assert popped is self._sem_poison
        nc.clear_and_free_semaphores(list(self.sems.allocated().values()))
        nc.all_engine_barrier()


# --------------------------------------------------------------------------
# host-side sharding / packing
# --------------------------------------------------------------------------

def _shard_and_pack(entity_indices):
    Nn = entity_indices.shape[0]
    starts = np.searchsorted(entity_indices, np.arange(E + 1))
    ideal = (np.arange(1, N_CORES) * Nn) // N_CORES
    ent_bnd = [0]
    for t in ideal:
        s = int(np.searchsorted(starts, t))
        if s > 0 and abs(int(starts[s - 1]) - int(t)) < abs(int(starts[s]) - int(t)):
            s -= 1
        ent_bnd.append(s)
    ent_bnd.append(E)

    cores = []
    for c in range(N_CORES):
        e_lo, e_hi = ent_bnd[c], ent_bnd[c + 1]
        segs = np.arange(e_lo, e_hi)
        sizes = (starts[e_lo + 1 : e_hi + 1] - starts[e_lo:e_hi]).astype(np.int64)
        n_edges = int(sizes.sum())
        assert e_hi - e_lo <= E_PAD and n_edges <= NL
        order = np.argsort(-sizes, kind="stable")
        blk_edges = [0] * NBLK
        blk_nseg = [0] * NBLK
        blk_segs = [[] for _ in range(NBLK)]
        for idx in order:
            sz = int(sizes[idx])
            best = -1
            for b in sorted(range(NBLK), key=lambda b: blk_edges[b]):
                if blk_nseg[b] < SPB and blk_edges[b] + sz <= EPB:
                    best = b
                    break
            assert best >= 0, "block packing overflow"
            blk_segs[best].append(int(segs[idx]))
            blk_edges[best] += sz
            blk_nseg[best] += 1
        perm = np.full(NL, -1, dtype=np.int64)
        seg_local = np.full(NL, PAD_SEG, dtype=np.float32)
        row2seg = np.full(E_PAD, -1, dtype=np.int64)
        inv_cnt = np.zeros(E_PAD, dtype=np.float32)
        for b in range(NBLK):
            pos = b * EPB
            for j, s in enumerate(blk_segs[b]):
                row = b * SPB + j
                row2seg[row] = s
                n = int(starts[s + 1] - starts[s])
                if n > 0:
                    inv_cnt[row] = 1.0 / n
                perm[pos : pos + n] = np.arange(starts[s], starts[s + 1])
                seg_local[pos : pos + n] = float(j)
                pos += n
        cores.append(dict(perm=perm, seg_local=seg_local, row2seg=row2seg,
                          inv_cnt=inv_cnt))
    return cores


# --------------------------------------------------------------------------
# device kernel
# --------------------------------------------------------------------------

def _build_nc():
    nc = bass.Bass("TRN2", target_bir_lowering=False, debug=False,
                   num_devices=N_CORES)

    f32, bf, i32 = dt.float32, dt.bfloat16, dt.int32
    enr_d = nc.dram_tensor("enr", [NL, ED], bf, kind="ExternalInput")
    cnt_d = nc.dram_tensor("cnt", [NL, CW], bf, kind="ExternalInput")
    sp_d = nc.dram_tensor("sp", [NL, 2], f32, kind="ExternalInput")
    icnt_d = nc.dram_tensor("inv_cnt", [E_PAD], f32, kind="ExternalInput")
    wenr_d = nc.dram_tensor("wenr", [128, ED], bf, kind="ExternalInput")
    wtr_d = nc.dram_tensor("wtr", [5 * 128, 12 * OH], bf, kind="ExternalInput")
    wte_d = nc.dram_tensor("wte", [5 * 128, 6 * OH], bf, kind="ExternalInput")
    orel_d = nc.dram_tensor("orel", [5 * NBLK * 128, OH], bf,
                            kind="ExternalOutput")
    oent_d = nc.dram_tensor("oent", [5 * NBLK * 128, OH], bf,
                            kind="ExternalOutput")

    with _TileContextSplitDrain(nc) as tc, ExitStack() as es:
        const = es.enter_context(tc.tile_pool(name="const", bufs=1))
        accp = es.enter_context(tc.tile_pool(name="accp", bufs=1))

        # ---- constants ----
        iota_seg = const.tile([128, 128], bf)
        ident = const.tile([128, 128], bf)
        with tc.tile_pool(name="setup", bufs=1) as setup:
            iota_i = setup.tile([128, 128], i32)
            nc.gpsimd.iota(iota_i[:], pattern=[[1, 128]], base=0,
                           channel_multiplier=0)
            nc.vector.tensor_copy(iota_seg[:], iota_i[:])
            iota_ci = setup.tile([128, 1], i32)
            nc.gpsimd.iota(iota_ci[:], pattern=[[0, 1]], base=0,
                           channel_multiplier=1)
            iota_col = setup.tile([128, 1], f32)
            nc.vector.tensor_copy(iota_col[:], iota_ci[:])
            nc.vector.tensor_scalar(out=ident[:], in0=iota_seg[:],
                                    scalar1=iota_col[:],
                                    scalar2=None, op0=Alu.is_equal)

        wenr = const.tile([128, ED], bf)
        nc.sync.dma_start(wenr[:], wenr_d.ap())
        icnt_sb = const.tile([128, NBLK], f32)
        nc.sync.dma_start(
            icnt_sb[:], icnt_d.ap().rearrange("(b p) -> p b", p=128)
        )
        invd_sb = accp.tile([128, NBLK], f32)

        # resident transposed aggregates, one tile per (feat-chunk, block):
        # t 0-5 = relation, 6-11 = count emb  -> relcat (K=12 chunks)
        # t 0-5 of entT = entity              -> ent (K=6 chunks)
        relcatT = [[accp.tile([128, 128], bf, name=f"relcatT{t}_{b}",
                              tag=f"relcatT{t}_{b}") for b in range(NBLK)]
                   for t in range(12)]
        entT = [[accp.tile([128, 128], bf, name=f"entT{t}_{b}",
                           tag=f"entT{t}_{b}") for b in range(NBLK)]
                for t in range(6)]

        # ---- merged aggregation + projection (Tile interleaves by deps) ----
        HE = CH // 2  # 5 edges per partition per half-block
        with tc.tile_pool(name="edges", bufs=2) as edges, \
             tc.tile_pool(name="chunkp", bufs=2) as chunkp, \
             tc.tile_pool(name="evac", bufs=2) as evac, \
             tc.tile_pool(name="wpool", bufs=2) as wpool, \
             tc.tile_pool(name="outp", bufs=2) as outp, \
             tc.tile_pool(name="psagg", bufs=1, space="PSUM") as psagg, \
             tc.tile_pool(name="pp", bufs=2, space="PSUM") as pp:
            for b in range(NBLK):
                halves = []
                for hb in range(2):
                    r0 = b * EPB + hb * (EPB // 2)
                    r1 = r0 + EPB // 2
                    enrh = edges.tile([128, HE * ED], bf, tag="enrh")
                    nc.sync.dma_start(
                        enrh[:],
                        enr_d.ap()[r0:r1, :].rearrange("(p j) d -> p j d", j=HE),
                    )
                    cnth = edges.tile([128, HE * CW], bf, tag="cnth")
                    nc.sync.dma_start(
                        cnth[:],
                        cnt_d.ap()[r0:r1, :].rearrange("(p j) d -> p j d", j=HE),
                    )
                    sph = edges.tile([128, HE * 2], f32, tag="sph")
                    nc.sync.dma_start(
                        sph[:],
                        sp_d.ap()[r0:r1, :].rearrange("(p j) c -> p j c", j=HE),
                    )
                    halves.append((enrh, cnth, sph))

                # score + one-hot build for all 10 chunks (persist across
                # the two aggregation passes)
                ohs, ohxs = [], []
                for j in range(CH):
                    enrh, cnth, sph = halves[j // HE]
                    jj = j % HE
                    slc = sph[:, 2 * jj : 2 * jj + 1]
                    prc = sph[:, 2 * jj + 1 : 2 * jj + 2]
                    scr = chunkp.tile([128, ED], bf, tag="scr")
                    sa = chunkp.tile([128, 1], f32, tag=f"sa{j}")
                    nc.vector.scalar_tensor_tensor(
                        out=scr[:], in0=enrh[:, jj * ED : (jj + 1) * ED],
                        scalar=1.0, in1=wenr[:],
                        op0=Alu.mult, op1=Alu.mult, accum_out=sa[:])
                    ex_ = chunkp.tile([128, 1], f32, tag=f"ex{j}")
                    nc.scalar.activation(ex_[:], sa[:], Act.Exp, bias=prc)
                    oh = chunkp.tile([128, 128], bf, tag=f"oh{j}")
                    nc.vector.tensor_scalar(out=oh[:], in0=iota_seg[:],
                                            scalar1=slc,
                                            scalar2=None, op0=Alu.is_equal)
                    ohx = chunkp.tile([128, 128], bf, tag=f"ohx{j}")
                    nc.vector.tensor_scalar(out=ohx[:], in0=iota_seg[:],
                                            scalar1=slc,
                                            scalar2=ex_[:],
                                            op0=Alu.is_equal, op1=Alu.mult)
                    ohs.append(oh)
                    ohxs.append(ohx)

                relsb = evac.tile([128, D], bf, tag="relsb")
                cntsb = evac.tile([128, D], bf, tag="cntsb")
                entsb = evac.tile([128, D], bf, tag="entsb")

                # pass A: feature cols 0:512 (cnt col 0 is the ones column,
                # so psA_cnt[:,0] accumulates the softmax denominator)
                psA_rel = psagg.tile([128, 512], f32, tag="ps_rel")
                psA_cnt = psagg.tile([128, 512], f32, tag="ps_cnt")
                psA_ent = psagg.tile([128, 512], f32, tag="ps_ent")
                for j in range(CH):
                    enrh, cnth, sph = halves[j // HE]
                    jj = j % HE
                    ej = enrh[:, jj * ED : jj * ED + D]
                    rj = enrh[:, jj * ED + 2 * D : jj * ED + 3 * D]
                    cj = cnth[:, jj * CW : (jj + 1) * CW]
                    st, sp_ = (j == 0), (j == CH - 1)
                    nc.tensor.matmul(psA_rel[:], ohxs[j][:], rj[:, 0:512],
                                     start=st, stop=sp_)
                    nc.tensor.matmul(psA_cnt[:], ohxs[j][:], cj[:, 0:512],
                                     start=st, stop=sp_)
                    nc.tensor.matmul(psA_ent[:], ohs[j][:], ej[:, 0:512],
                                     start=st, stop=sp_)

                dmx = chunkp.tile([128, 1], f32, tag="dmx")
                nc.vector.tensor_scalar(out=dmx[:], in0=psA_cnt[:, 0:1],
                                        scalar1=1e-30, scalar2=None, op0=Alu.max)
                nc.vector.reciprocal(invd_sb[:, b : b + 1], dmx[:])
                nc.scalar.activation(relsb[:, 0:512], psA_rel[:], Act.Copy,
                                     scale=invd_sb[:, b : b + 1])
                nc.scalar.activation(cntsb[:, 0:511], psA_cnt[:, 1:512],
                                     Act.Copy, scale=invd_sb[:, b : b + 1])
                nc.scalar.activation(entsb[:, 0:512], psA_ent[:], Act.Copy,
                                     scale=icnt_sb[:, b : b + 1])

                # pass B: feature cols 512:768 (+ count tail)
                psB_rel = psagg.tile([128, 512], f32, tag="ps_rel")
                psB_cnt = psagg.tile([128, 512], f32, tag="ps_cnt")
                psB_ent = psagg.tile([128, 512], f32, tag="ps_ent")
                for j in range(CH):
                    enrh, cnth, sph = halves[j // HE]
                    jj = j % HE
                    ej = enrh[:, jj * ED : jj * ED + D]
                    rj = enrh[:, jj * ED + 2 * D : jj * ED + 3 * D]
                    cj = cnth[:, jj * CW : (jj + 1) * CW]
                    st, sp_ = (j == 0), (j == CH - 1)
                    nc.tensor.matmul(psB_rel[:, 0:256], ohxs[j][:],
                                     rj[:, 512:D], start=st, stop=sp_)
                    nc.tensor.matmul(psB_cnt[:, 0:264], ohxs[j][:],
                                     cj[:, 512:CW], start=st, stop=sp_)
                    nc.tensor.matmul(psB_ent[:, 0:256], ohs[j][:],
                                     ej[:, 512:D], start=st, stop=sp_)

                nc.scalar.activation(relsb[:, 512:D], psB_rel[:, 0:256],
                                     Act.Copy, scale=invd_sb[:, b : b + 1])
                nc.scalar.activation(cntsb[:, 511:D], psB_cnt[:, 0:257],
                                     Act.Copy, scale=invd_sb[:, b : b + 1])
                nc.scalar.activation(entsb[:, 512:D], psB_ent[:, 0:256],
                                     Act.Copy, scale=icnt_sb[:, b : b + 1])

                for t in range(6):
                    pt = pp.tile([128, 128], bf, tag="pt")
                    nc.tensor.transpose(pt[:],
                                        relsb[:, t * 128 : (t + 1) * 128],
                                        ident[:])
                    nc.scalar.activation(relcatT[t][b][:], pt[:], Act.Copy)
                    pt2 = pp.tile([128, 128], bf, tag="pt")
                    nc.tensor.transpose(pt2[:],
                                        cntsb[:, t * 128 : (t + 1) * 128],
                                        ident[:])
                    nc.scalar.activation(relcatT[6 + t][b][:], pt2[:],
                                         Act.Copy)
                    pt3 = pp.tile([128, 128], bf, tag="pt")
                    nc.tensor.transpose(pt3[:],
                                        entsb[:, t * 128 : (t + 1) * 128],
                                        ident[:])
                    nc.scalar.activation(entT[t][b][:], pt3[:], Act.Copy)

            # ---- projections (interleave with later aggregation blocks) ----
            for (Tt, wt_d, o_d, KC) in (
                (relcatT, wtr_d, orel_d, 12),
                (entT, wte_d, oent_d, 6),
            ):
                for h in range(5):
                    wt = wpool.tile([128, KC * OH], bf, tag="wt")
                    nc.gpsimd.dma_start(
                        wt[:], wt_d.ap()[h * 128 : (h + 1) * 128, :])
                    for sblk in range(NBLK):
                        stage = outp.tile([128, OH], bf, tag="stage")
                        for oc5 in range(OH // 512):
                            pso = pp.tile([128, 512], f32, tag="pso", bufs=3)
                            for k in range(KC):
                                nc.tensor.matmul(
                                    pso[:],
                                    Tt[k][sblk][:],
                                    wt[:, k * OH + oc5 * 512 : k * OH + oc5 * 512 + 512],
                                    start=(k == 0), stop=(k == KC - 1))
                            if oc5 % 2 == 0:
                                nc.vector.tensor_copy(
                                    stage[:, oc5 * 512 : (oc5 + 1) * 512], pso[:])
                            else:
                                nc.scalar.activation(
                                    stage[:, oc5 * 512 : (oc5 + 1) * 512], pso[:],
                                    Act.Copy)
                        nc.gpsimd.dma_start(
                            o_d.ap()[(h * NBLK + sblk) * 128 :
                                     (h * NBLK + sblk + 1) * 128, :],
                            stage[:],
                        )
    return nc


_NC_CACHE = None


def _get_nc():
    global _NC_CACHE
    if _NC_CACHE is None:
        _NC_CACHE = _build_nc()
    return _NC_CACHE


# --------------------------------------------------------------------------
# entry point
# --------------------------------------------------------------------------

def kernel(prompt_embs, entity_embs, neighbor_embs, relation_embs,
           count_table, scorer_W, scorer_b, rel_W, rel_b, ent_W, ent_b,
           counts, prompt_indices, entity_indices):
    from concourse.bass_utils import run_bass_kernel_spmd

    prompt_embs = np.asarray(prompt_embs, dtype=np.float32)
    entity_embs = np.asarray(entity_embs, dtype=np.float32)
    neighbor_embs = np.asarray(neighbor_embs, dtype=np.float32)
    relation_embs = np.asarray(relation_embs, dtype=np.float32)
    count_table = np.asarray(count_table, dtype=np.float32)
    scorer_W = np.asarray(scorer_W, dtype=np.float32)
    scorer_b = np.asarray(scorer_b, dtype=np.float32)
    rel_W = np.asarray(rel_W, dtype=np.float32)
    rel_b = np.asarray(rel_b, dtype=np.float32)
    ent_W = np.asarray(ent_W, dtype=np.float32)
    ent_b = np.asarray(ent_b, dtype=np.float32)
    counts = np.asarray(counts)
    prompt_indices = np.asarray(prompt_indices)
    entity_indices = np.asarray(entity_indices)

    cores = _shard_and_pack(entity_indices)

    # replicated (weight-derived) host prep
    w = scorer_W[0]
    pscore = (prompt_embs * w[None, :D]).sum(1) + scorer_b[0]     # fold bias
    cscore = (count_table * w[None, 4 * D :]).sum(1)
    wenr = np.broadcast_to(w[D : 4 * D].astype(BF16), (128, ED)).copy()

    # merged bf16 edge features [ent|nbr|rel] and padded count embs
    enr_full = np.empty((N, ED), BF16)
    enr_full[:, 0:D] = entity_embs.astype(BF16)
    enr_full[:, D : 2 * D] = neighbor_embs.astype(BF16)
    enr_full[:, 2 * D : 3 * D] = relation_embs.astype(BF16)
    ct_bf = count_table.astype(BF16)
    # per-edge prescore = prompt score + count score (+ bias)
    pres_full = (pscore[prompt_indices] + cscore[counts]).astype(np.float32)

    # pre-tiled projection weights: [h*128+p, k*OH+c] = W[h*OH+c, k*128+p]
    def tile_w(W, KC):
        WT = np.ascontiguousarray(W.T).astype(BF16)          # [K*128, OUT]
        return np.ascontiguousarray(
            WT.reshape(KC, 128, 5, OH).transpose(2, 1, 0, 3)
        ).reshape(5 * 128, KC * OH)

    wtr = tile_w(rel_W, 12)
    wte = tile_w(ent_W, 6)

    in_maps = []
    for core in cores:
        perm = core["perm"]
        valid = perm >= 0
        src = np.where(valid, perm, 0)

        enr = enr_full[src]
        enr[~valid] = 0
        cnt = np.zeros((NL, CW), BF16)
        cnt[:, 0] = 1.0          # ones col -> softmax denominator (pass A)
        cnt[~valid, 0] = 0
        cnt[:, 1 : D + 1] = ct_bf[counts[src]]
        cnt[~valid, 1 : D + 1] = 0
        sp = np.zeros((NL, 2), np.float32)
        sp[:, 0] = core["seg_local"]
        sp[:, 1] = pres_full[src]
        sp[~valid, 1] = 0.0

        in_maps.append(dict(
            enr=np.ascontiguousarray(enr), cnt=np.ascontiguousarray(cnt),
            sp=sp, inv_cnt=core["inv_cnt"], wenr=wenr,
            wtr=wtr, wte=wte,
        ))

    nc = _get_nc()
    res = run_bass_kernel_spmd(nc, in_maps, list(range(N_CORES)))

    rel_out = np.zeros((E, OUT), np.float32)
    ent_out = np.zeros((E, OUT), np.float32)
    for c, core in enumerate(cores):
        rows = core["row2seg"]
        mask = rows >= 0
        # output DRAM layout [5h x 10blk x 128p, 1024c] -> [1280, 5120]
        orel = np.asarray(res.results[c]["orel"], dtype=np.float32)
        oent = np.asarray(res.results[c]["oent"], dtype=np.float32)
        orel = orel.reshape(5, NBLK * 128, OH).transpose(1, 0, 2).reshape(E_PAD, OUT)
        oent = oent.reshape(5, NBLK * 128, OH).transpose(1, 0, 2).reshape(E_PAD, OUT)
        rel_out[rows[mask]] = orel[mask]
        ent_out[rows[mask]] = oent[mask]
    rel_out += rel_b[None, :]
    ent_out += ent_b[None, :]
    return rel_out, ent_out


# revision 8
# speedup vs baseline: 1.3758x; 1.0395x over previous
"""EntityEncoder (gnn_message_passing) Trainium2 kernel — 8-core SPMD.

Strategy: edges are pre-partitioned on the host into 8 contiguous,
entity-aligned, edge-balanced shards (entity_indices is sorted, so each
entity's edges land wholly on one core — no cross-core collectives).
Within a core, segments are LPT-packed into 10 blocks of <=128 segments /
<=1280 edges.  All HBM traffic is bf16.  The host folds the prompt-score,
count-score and scorer bias into a per-edge prescore, gathers per-edge
count embeddings (with an appended ones column that yields the softmax
denominator for free), and pre-tiles the projection weights.  On device:
one fused 2304-col dot per 128-edge chunk (vector), exp on scalar,
one-hot segment matmuls on tensor for the three segment reductions,
PE transposes of the [seg,feat] aggregates, then bf16 output projections.
Projection bias and the final row scatter are applied on the host.
"""
import sys
import numpy as np
import ml_dtypes

for _p in ("/root/.axon_site", "/root/.axon_site/_ro/trn_rl_repo",
           "/root/.axon_site/_ro/pypackages"):
    if _p not in sys.path:
        sys.path.append(_p)

import bass_rust
import concourse.bass as bass
import concourse.mybir as mybir
import concourse.tile as tile
from concourse.vector_clock import ScopedClock
from contextlib import ExitStack

BF16 = ml_dtypes.bfloat16
dt = mybir.dt
Alu = mybir.AluOpType
Act = mybir.ActivationFunctionType

# problem shape (hardcoded per contest contract)
N_CORES = 8
N = 100_000
P = 64
E = 10_000
D = 768
C = 1000
OUT = 5120
# per-core packing
NBLK = 10
SPB = 128                # segs per block
CH = 10                  # chunks (of 128 edges) per block
EPB = CH * 128           # edges per block = 1280
NL = NBLK * EPB          # 12800 edge slots per core
E_PAD = NBLK * SPB       # 1280 seg slots per core
OH = OUT // 5            # 1024-wide output slab
PAD_SEG = 999.0
CW = 776                 # count-emb cols: 768 + ones col + 7 pad
ED = 3 * D               # merged [ent|nbr|rel] width = 2304


class _TileContextSplitDrain(tile.TileContext):
    """This container's walrus accepts only ONE sync wait per instruction
    ("Too many sync wait commands" in setupSyncWait). Split every extra wait
    onto a standalone same-engine NoOp placed immediately before the
    instruction — identical semantics, one wait per instruction."""

    def _lower_ordered_insts(self, ordered):
        for insts in ordered.values():
            if not any(
                i.sync_info is not None and len(i.sync_info.on_wait) > 1
                for i in insts
            ):
                continue
            new = []
            for inst in insts:
                si = inst.sync_info
                if si is not None and len(si.on_wait) > 1:
                    waits = list(si.on_wait)
                    for w in waits[:-1]:
                        nop = bass_rust.InstNoOp(
                            name=self.nc.get_next_instruction_name(),
                            ins=[], outs=[])
                        nop.engine = inst.engine
                        nop.sync_info = bass_rust.SyncInfo(
                            on_wait=[w], on_update=[])
                        new.append(nop)
                    si.on_wait = waits[-1:]
                new.append(inst)
            insts[:] = new
        return super()._lower_ordered_insts(ordered)

    def _drain_and_barrier(self, tick_clock, wait_clock):
        nc = self.nc
        drain_inst = nc.sync.drain()
        wait_clock.add_sem_waits(
            drain_inst.ins, ScopedClock({None: tick_clock.global_clock})
        )
        si = drain_inst.ins.sync_info
        if si is not None and len(si.on_wait) > 1:
            waits = list(si.on_wait)
            si.on_wait = waits[:1]
            for w in waits[1:]:
                n = nc.sync.nop()
                n.ins.sync_info = bass_rust.SyncInfo(on_wait=[w], on_update=[])
        nc.all_engine_barrier()
        assert self.sems is not None
        popped = nc._tile_sem_poison_stack.pop()
        assert popped is self._sem_poison
        nc.clear_and_free_semaphores(list(self.sems.allocated().values()))
        nc.all_engine_barrier()


# --------------------------------------------------------------------------
# host-side sharding / packing
# --------------------------------------------------------------------------

def _shard_and_pack(entity_indices):
    Nn = entity_indices.shape[0]
    starts = np.searchsorted(entity_indices, np.arange(E + 1))
    ideal = (np.arange(1, N_CORES) * Nn) // N_CORES
    ent_bnd = [0]
    for t in ideal:
        s = int(np.searchsorted(starts, t))
        if s > 0 and abs(int(starts[s - 1]) - int(t)) < abs(int(starts[s]) - int(t)):
            s -= 1
        ent_bnd.append(s)
    ent_bnd.append(E)

    cores = []
    for c in range(N_CORES):
        e_lo, e_hi = ent_bnd[c], ent_bnd[c + 1]
        segs = np.arange(e_lo, e_hi)
        sizes = (starts[e_lo + 1 : e_hi + 1] - starts[e_lo:e_hi]).astype(np.int64)
        n_edges = int(sizes.sum())
        assert e_hi - e_lo <= E_PAD and n_edges <= NL
        order = np.argsort(-sizes, kind="stable")
        blk_edges = [0] * NBLK
        blk_nseg = [0] * NBLK
        blk_segs = [[] for _ in range(NBLK)]
        for idx in order:
            sz = int(sizes[idx])
            best = -1
            for b in sorted(range(NBLK), key=lambda b: blk_edges[b]):
                if blk_nseg[b] < SPB and blk_edges[b] + sz <= EPB:
                    best = b
                    break
            assert best >= 0, "block packing overflow"
            blk_segs[best].append(int(segs[idx]))
            blk_edges[best] += sz
            blk_nseg[best] += 1
        perm = np.full(NL, -1, dtype=np.int64)
        seg_local = np.full(NL, PAD_SEG, dtype=np.float32)
        row2seg = np.full(E_PAD, -1, dtype=np.int64)
        inv_cnt = np.zeros(E_PAD, dtype=np.float32)
        for b in range(NBLK):
            pos = b * EPB
            for j, s in enumerate(blk_segs[b]):
                row = b * SPB + j
                row2seg[row] = s
                n = int(starts[s + 1] - starts[s])
                if n > 0:
                    inv_cnt[row] = 1.0 / n
                perm[pos : pos + n] = np.arange(starts[s], starts[s + 1])
                seg_local[pos : pos + n] = float(j)
                pos += n
        cores.append(dict(perm=perm, seg_local=seg_local, row2seg=row2seg,
                          inv_cnt=inv_cnt))
    return cores


# --------------------------------------------------------------------------
# device kernel
# --------------------------------------------------------------------------

def _build_nc():
    nc = bass.Bass("TRN2", target_bir_lowering=False, debug=False,
                   num_devices=N_CORES)

    f32, bf, i32 = dt.float32, dt.bfloat16, dt.int32
    enr_d = nc.dram_tensor("enr", [NL, ED], bf, kind="ExternalInput")
    cnt_d = nc.dram_tensor("cnt", [NL, CW], bf, kind="ExternalInput")
    sp_d = nc.dram_tensor("sp", [NL, 2], f32, kind="ExternalInput")
    icnt_d = nc.dram_tensor("inv_cnt", [E_PAD], f32, kind="ExternalInput")
    wenr_d = nc.dram_tensor("wenr", [128, ED], bf, kind="ExternalInput")
    wtr_d = nc.dram_tensor("wtr", [5 * 128, 12 * OH], bf, kind="ExternalInput")
    wte_d = nc.dram_tensor("wte", [5 * 128, 6 * OH], bf, kind="ExternalInput")
    orel_d = nc.dram_tensor("orel", [5 * NBLK * 128, OH], bf,
                            kind="ExternalOutput")
    oent_d = nc.dram_tensor("oent", [5 * NBLK * 128, OH], bf,
                            kind="ExternalOutput")

    with _TileContextSplitDrain(nc) as tc, ExitStack() as es:
        const = es.enter_context(tc.tile_pool(name="const", bufs=1))
        accp = es.enter_context(tc.tile_pool(name="accp", bufs=1))

        # ---- constants ----
        iota_seg = const.tile([128, 128], bf)
        ident = const.tile([128, 128], bf)
        with tc.tile_pool(name="setup", bufs=1) as setup:
            iota_i = setup.tile([128, 128], i32)
            nc.gpsimd.iota(iota_i[:], pattern=[[1, 128]], base=0,
                           channel_multiplier=0)
            nc.vector.tensor_copy(iota_seg[:], iota_i[:])
            iota_ci = setup.tile([128, 1], i32)
            nc.gpsimd.iota(iota_ci[:], pattern=[[0, 1]], base=0,
                           channel_multiplier=1)
            iota_col = setup.tile([128, 1], f32)
            nc.vector.tensor_copy(iota_col[:], iota_ci[:])
            nc.vector.tensor_scalar(out=ident[:], in0=iota_seg[:],
                                    scalar1=iota_col[:],
                                    scalar2=None, op0=Alu.is_equal)

        wenr = const.tile([128, ED], bf)
        nc.sync.dma_start(wenr[:], wenr_d.ap())
        icnt_sb = const.tile([128, NBLK], f32)
        nc.sync.dma_start(
            icnt_sb[:], icnt_d.ap().rearrange("(b p) -> p b", p=128)
        )
        invd_sb = accp.tile([128, NBLK], f32)

        # resident transposed aggregates, one tile per (feat-chunk, block):
        # t 0-5 = relation, 6-11 = count emb  -> relcat (K=12 chunks)
        # t 0-5 of entT = entity              -> ent (K=6 chunks)
        relcatT = [[accp.tile([128, 128], bf, name=f"relcatT{t}_{b}",
                              tag=f"relcatT{t}_{b}") for b in range(NBLK)]
                   for t in range(12)]
        entT = [[accp.tile([128, 128], bf, name=f"entT{t}_{b}",
                           tag=f"entT{t}_{b}") for b in range(NBLK)]
                for t in range(6)]

        # ---- merged aggregation + projection (Tile interleaves by deps) ----
        HE = CH // 2  # 5 edges per partition per half-block
        with tc.tile_pool(name="edges", bufs=2) as edges, \
             tc.tile_pool(name="chunkp", bufs=2) as chunkp, \
             tc.tile_pool(name="evac", bufs=2) as evac, \
             tc.tile_pool(name="wpool", bufs=2) as wpool, \
             tc.tile_pool(name="outp", bufs=2) as outp, \
             tc.tile_pool(name="psagg", bufs=1, space="PSUM") as psagg, \
             tc.tile_pool(name="pp", bufs=2, space="PSUM") as pp:
            for b in range(NBLK):
                halves = []
                for hb in range(2):
                    r0 = b * EPB + hb * (EPB // 2)
                    r1 = r0 + EPB // 2
                    enrh = edges.tile([128, HE * ED], bf, tag="enrh")
                    nc.sync.dma_start(
                        enrh[:],
                        enr_d.ap()[r0:r1, :].rearrange("(p j) d -> p j d", j=HE),
                    )
                    cnth = edges.tile([128, HE * CW], bf, tag="cnth")
                    nc.sync.dma_start(
                        cnth[:],
                        cnt_d.ap()[r0:r1, :].rearrange("(p j) d -> p j d", j=HE),
                    )
                    sph = edges.tile([128, HE * 2], f32, tag="sph")
                    nc.sync.dma_start(
                        sph[:],
                        sp_d.ap()[r0:r1, :].rearrange("(p j) c -> p j c", j=HE),
                    )
                    halves.append((enrh, cnth, sph))

                # score + one-hot build for all 10 chunks (persist across
                # the two aggregation passes)
                ohs, ohxs = [], []
                for j in range(CH):
                    enrh, cnth, sph = halves[j // HE]
                    jj = j % HE
                    slc = sph[:, 2 * jj : 2 * jj + 1]
                    prc = sph[:, 2 * jj + 1 : 2 * jj + 2]
                    scr = chunkp.tile([128, ED], bf, tag="scr")
                    sa = chunkp.tile([128, 1], f32, tag=f"sa{j}")
                    nc.vector.scalar_tensor_tensor(
                        out=scr[:], in0=enrh[:, jj * ED : (jj + 1) * ED],
                        scalar=1.0, in1=wenr[:],
                        op0=Alu.mult, op1=Alu.mult, accum_out=sa[:])
                    ex_ = chunkp.tile([128, 1], f32, tag=f"ex{j}")
                    nc.scalar.activation(ex_[:], sa[:], Act.Exp, bias=prc)
                    oh = chunkp.tile([128, 128], bf, tag=f"oh{j}")
                    nc.vector.tensor_scalar(out=oh[:], in0=iota_seg[:],
                                            scalar1=slc,
                                            scalar2=None, op0=Alu.is_equal)
                    ohx = chunkp.tile([128, 128], bf, tag=f"ohx{j}")
                    nc.vector.tensor_scalar(out=ohx[:], in0=iota_seg[:],
                                            scalar1=slc,
                                            scalar2=ex_[:],
                                            op0=Alu.is_equal, op1=Alu.mult)
                    ohs.append(oh)
                    ohxs.append(ohx)

                relsb = evac.tile([128, D], bf, tag="relsb")
                cntsb = evac.tile([128, D], bf, tag="cntsb")
                entsb = evac.tile([128, D], bf, tag="entsb")

                # pass A: feature cols 0:512 (cnt col 0 is the ones column,
                # so psA_cnt[:,0] accumulates the softmax denominator)
                psA_rel = psagg.tile([128, 512], f32, tag="ps_rel")
                psA_cnt = psagg.tile([128, 512], f32, tag="ps_cnt")
                psA_ent = psagg.tile([128, 512], f32, tag="ps_ent")
                for j in range(CH):
                    enrh, cnth, sph = halves[j // HE]
                    jj = j % HE
                    ej = enrh[:, jj * ED : jj * ED + D]
                    rj = enrh[:, jj * ED + 2 * D : jj * ED + 3 * D]
                    cj = cnth[:, jj * CW : (jj + 1) * CW]
                    st, sp_ = (j == 0), (j == CH - 1)
                    nc.tensor.matmul(psA_rel[:], ohxs[j][:], rj[:, 0:512],
                                     start=st, stop=sp_)
                    nc.tensor.matmul(psA_cnt[:], ohxs[j][:], cj[:, 0:512],
                                     start=st, stop=sp_)
                    nc.tensor.matmul(psA_ent[:], ohs[j][:], ej[:, 0:512],
                                     start=st, stop=sp_)

                dmx = chunkp.tile([128, 1], f32, tag="dmx")
                nc.vector.tensor_scalar(out=dmx[:], in0=psA_cnt[:, 0:1],
                                        scalar1=1e-30, scalar2=None, op0=Alu.max)
                nc.vector.reciprocal(invd_sb[:, b : b + 1], dmx[:])
                nc.scalar.activation(relsb[:, 0:512], psA_rel[:], Act.Copy,
                                     scale=invd_sb[:, b : b + 1])
                nc.scalar.activation(cntsb[:, 0:511], psA_cnt[:, 1:512],
                                     Act.Copy, scale=invd_sb[:, b : b + 1])
                nc.scalar.activation(entsb[:, 0:512], psA_ent[:], Act.Copy,
                                     scale=icnt_sb[:, b : b + 1])

                # pass B: feature cols 512:768 (+ count tail)
                psB_rel = psagg.tile([128, 512], f32, tag="ps_rel")
                psB_cnt = psagg.tile([128, 512], f32, tag="ps_cnt")
                psB_ent = psagg.tile([128, 512], f32, tag="ps_ent")
                for j in range(CH):
                    enrh, cnth, sph = halves[j // HE]
                    jj = j % HE
                    ej = enrh[:, jj * ED : jj * ED + D]
                    rj = enrh[:, jj * ED + 2 * D : jj * ED + 3 * D]
                    cj = cnth[:, jj * CW : (jj + 1) * CW]
                    st, sp_ = (j == 0), (j == CH - 1)
                    nc.tensor.matmul(psB_rel[:, 0:256], ohxs[j][:],
                                     rj[:, 512:D], start=st, stop=sp_)
                    nc.tensor.matmul(psB_cnt[:, 0:264], ohxs[j][:],
                                     cj[:, 512:CW], start=st, stop=sp_)
                    nc.tensor.matmul(psB_ent[:, 0:256], ohs[j][:],
                                     ej[:, 512:D], start=st, stop=sp_)

                nc.scalar.activation(relsb[:, 512:D], psB_rel[:, 0:256],
                                     Act.Copy, scale=invd_sb[:, b : b + 1])
                nc.scalar.activation(cntsb[:, 511:D], psB_cnt[:, 0:257],
                                     Act.Copy, scale=invd_sb[:, b : b + 1])
                nc.scalar.activation(entsb[:, 512:D], psB_ent[:, 0:256],
                                     Act.Copy, scale=icnt_sb[:, b : b + 1])

                for t in range(6):
                    pt = pp.tile([128, 128], bf, tag="pt")
                    nc.tensor.transpose(pt[:],
                                        relsb[:, t * 128 : (t + 1) * 128],
                                        ident[:])
                    nc.scalar.activation(relcatT[t][b][:], pt[:], Act.Copy)
                    pt2 = pp.tile([128, 128], bf, tag="pt")
                    nc.tensor.transpose(pt2[:],
                                        cntsb[:, t * 128 : (t + 1) * 128],
                                        ident[:])
                    nc.scalar.activation(relcatT[6 + t][b][:], pt2[:],
                                         Act.Copy)
                    pt3 = pp.tile([128, 128], bf, tag="pt")
                    nc.tensor.transpose(pt3[:],
                                        entsb[:, t * 128 : (t + 1) * 128],
                                        ident[:])
                    nc.scalar.activation(entT[t][b][:], pt3[:], Act.Copy)

            # ---- projections (interleave with later aggregation blocks) ----
            for (Tt, wt_d, o_d, KC) in (
                (relcatT, wtr_d, orel_d, 12),
                (entT, wte_d, oent_d, 6),
            ):
                for h in range(5):
                    wt = wpool.tile([128, KC * OH], bf, tag="wt")
                    nc.gpsimd.dma_start(
                        wt[:], wt_d.ap()[h * 128 : (h + 1) * 128, :])
                    for sblk in range(NBLK):
                        stage = outp.tile([128, OH], bf, tag="stage", bufs=4)
                        for oc5 in range(OH // 512):
                            pso = pp.tile([128, 512], f32, tag="pso", bufs=3)
                            for k in range(KC):
                                nc.tensor.matmul(
                                    pso[:],
                                    Tt[k][sblk][:],
                                    wt[:, k * OH + oc5 * 512 : k * OH + oc5 * 512 + 512],
                                    start=(k == 0), stop=(k == KC - 1))
                            if oc5 % 2 == 0:
                                nc.vector.tensor_copy(
                                    stage[:, oc5 * 512 : (oc5 + 1) * 512], pso[:])
                            else:
                                nc.scalar.activation(
                                    stage[:, oc5 * 512 : (oc5 + 1) * 512], pso[:],
                                    Act.Copy)
                        nc.scalar.dma_start(
                            o_d.ap()[(h * NBLK + sblk) * 128 :
                                     (h * NBLK + sblk + 1) * 128, :],
                            stage[:],
                        )
    return nc


_NC_CACHE = None


def _get_nc():
    global _NC_CACHE
    if _NC_CACHE is None:
        _NC_CACHE = _build_nc()
    return _NC_CACHE


# --------------------------------------------------------------------------
# entry point
# --------------------------------------------------------------------------

def kernel(prompt_embs, entity_embs, neighbor_embs, relation_embs,
           count_table, scorer_W, scorer_b, rel_W, rel_b, ent_W, ent_b,
           counts, prompt_indices, entity_indices):
    from concourse.bass_utils import run_bass_kernel_spmd

    prompt_embs = np.asarray(prompt_embs, dtype=np.float32)
    entity_embs = np.asarray(entity_embs, dtype=np.float32)
    neighbor_embs = np.asarray(neighbor_embs, dtype=np.float32)
    relation_embs = np.asarray(relation_embs, dtype=np.float32)
    count_table = np.asarray(count_table, dtype=np.float32)
    scorer_W = np.asarray(scorer_W, dtype=np.float32)
    scorer_b = np.asarray(scorer_b, dtype=np.float32)
    rel_W = np.asarray(rel_W, dtype=np.float32)
    rel_b = np.asarray(rel_b, dtype=np.float32)
    ent_W = np.asarray(ent_W, dtype=np.float32)
    ent_b = np.asarray(ent_b, dtype=np.float32)
    counts = np.asarray(counts)
    prompt_indices = np.asarray(prompt_indices)
    entity_indices = np.asarray(entity_indices)

    cores = _shard_and_pack(entity_indices)

    # replicated (weight-derived) host prep
    w = scorer_W[0]
    pscore = (prompt_embs * w[None, :D]).sum(1) + scorer_b[0]     # fold bias
    cscore = (count_table * w[None, 4 * D :]).sum(1)
    wenr = np.broadcast_to(w[D : 4 * D].astype(BF16), (128, ED)).copy()

    # merged bf16 edge features [ent|nbr|rel] and padded count embs
    enr_full = np.empty((N, ED), BF16)
    enr_full[:, 0:D] = entity_embs.astype(BF16)
    enr_full[:, D : 2 * D] = neighbor_embs.astype(BF16)
    enr_full[:, 2 * D : 3 * D] = relation_embs.astype(BF16)
    ct_bf = count_table.astype(BF16)
    # per-edge prescore = prompt score + count score (+ bias)
    pres_full = (pscore[prompt_indices] + cscore[counts]).astype(np.float32)

    # pre-tiled projection weights: [h*128+p, k*OH+c] = W[h*OH+c, k*128+p]
    def tile_w(W, KC):
        WT = np.ascontiguousarray(W.T).astype(BF16)          # [K*128, OUT]
        return np.ascontiguousarray(
            WT.reshape(KC, 128, 5, OH).transpose(2, 1, 0, 3)
        ).reshape(5 * 128, KC * OH)

    wtr = tile_w(rel_W, 12)
    wte = tile_w(ent_W, 6)

    in_maps = []
    for core in cores:
        perm = core["perm"]
        valid = perm >= 0
        src = np.where(valid, perm, 0)

        enr = enr_full[src]
        enr[~valid] = 0
        cnt = np.zeros((NL, CW), BF16)
        cnt[:, 0] = 1.0          # ones col -> softmax denominator (pass A)
        cnt[~valid, 0] = 0
        cnt[:, 1 : D + 1] = ct_bf[counts[src]]
        cnt[~valid, 1 : D + 1] = 0
        sp = np.zeros((NL, 2), np.float32)
        sp[:, 0] = core["seg_local"]
        sp[:, 1] = pres_full[src]
        sp[~valid, 1] = 0.0

        in_maps.append(dict(
            enr=np.ascontiguousarray(enr), cnt=np.ascontiguousarray(cnt),
            sp=sp, inv_cnt=core["inv_cnt"], wenr=wenr,
            wtr=wtr, wte=wte,
        ))

    nc = _get_nc()
    res = run_bass_kernel_spmd(nc, in_maps, list(range(N_CORES)))

    rel_out = np.zeros((E, OUT), np.float32)
    ent_out = np.zeros((E, OUT), np.float32)
    for c, core in enumerate(cores):
        rows = core["row2seg"]
        mask = rows >= 0
        # output DRAM layout [5h x 10blk x 128p, 1024c] -> [1280, 5120]
        orel = np.asarray(res.results[c]["orel"], dtype=np.float32)
        oent = np.asarray(res.results[c]["oent"], dtype=np.float32)
        orel = orel.reshape(5, NBLK * 128, OH).transpose(1, 0, 2).reshape(E_PAD, OUT)
        oent = oent.reshape(5, NBLK * 128, OH).transpose(1, 0, 2).reshape(E_PAD, OUT)
        rel_out[rows[mask]] = orel[mask]
        ent_out[rows[mask]] = oent[mask]
    rel_out += rel_b[None, :]
    ent_out += ent_b[None, :]
    return rel_out, ent_out


# revision 19
# speedup vs baseline: 1.6962x; 1.2329x over previous
"""EntityEncoder (gnn_message_passing) Trainium2 kernel — 8-core SPMD.

Strategy: edges are pre-partitioned on the host into 8 contiguous,
entity-aligned, edge-balanced shards (entity_indices is sorted, so each
entity's edges land wholly on one core — no cross-core collectives).
Within a core, segments are LPT-packed into 10 blocks of <=128 segments /
<=1280 edges.  All HBM traffic is bf16.  The host folds the prompt-score,
count-score and scorer bias into a per-edge prescore, gathers per-edge
count embeddings (with an appended ones column that yields the softmax
denominator for free), and pre-tiles the projection weights.  On device:
one fused 2304-col dot per 128-edge chunk (vector), exp on scalar,
one-hot segment matmuls on tensor for the three segment reductions,
PE transposes of the [seg,feat] aggregates, then bf16 output projections.
Projection bias and the final row scatter are applied on the host.
"""
import sys
import numpy as np
import ml_dtypes

for _p in ("/root/.axon_site", "/root/.axon_site/_ro/trn_rl_repo",
           "/root/.axon_site/_ro/pypackages"):
    if _p not in sys.path:
        sys.path.append(_p)

import bass_rust
import concourse.bass as bass
import concourse.mybir as mybir
import concourse.tile as tile
from concourse.vector_clock import ScopedClock
from contextlib import ExitStack

BF16 = ml_dtypes.bfloat16
dt = mybir.dt
Alu = mybir.AluOpType
Act = mybir.ActivationFunctionType

# problem shape (hardcoded per contest contract)
N_CORES = 8
N = 100_000
P = 64
E = 10_000
D = 768
C = 1000
OUT = 5120
# per-core packing
NBLK = 10
SPB = 128                # segs per block
CH = 10                  # chunks (of 128 edges) per block
EPB = CH * 128           # edges per block = 1280
NL = NBLK * EPB          # 12800 edge slots per core
E_PAD = NBLK * SPB       # 1280 seg slots per core
OH = OUT // 5            # 1024-wide output slab
PAD_SEG = 999.0
EDA = 1024               # pass-A edge cols: ent0(512) + rel0(512)
EDB = 512                # pass-B edge cols: ent1(256) + rel1(256)
CWB = 264                # pass-B count cols: ce[511:768] + 7 pad


class _TileContextSplitDrain(tile.TileContext):
    """This container's walrus accepts only ONE sync wait per instruction
    ("Too many sync wait commands" in setupSyncWait). Split every extra wait
    onto a standalone same-engine NoOp placed immediately before the
    instruction — identical semantics, one wait per instruction."""

    def _lower_ordered_insts(self, ordered):
        for insts in ordered.values():
            if not any(
                i.sync_info is not None and len(i.sync_info.on_wait) > 1
                for i in insts
            ):
                continue
            new = []
            for inst in insts:
                si = inst.sync_info
                if si is not None and len(si.on_wait) > 1:
                    waits = list(si.on_wait)
                    for w in waits[:-1]:
                        nop = bass_rust.InstNoOp(
                            name=self.nc.get_next_instruction_name(),
                            ins=[], outs=[])
                        nop.engine = inst.engine
                        nop.sync_info = bass_rust.SyncInfo(
                            on_wait=[w], on_update=[])
                        new.append(nop)
                    si.on_wait = waits[-1:]
                new.append(inst)
            insts[:] = new
        return super()._lower_ordered_insts(ordered)

    def _drain_and_barrier(self, tick_clock, wait_clock):
        nc = self.nc
        drain_inst = nc.sync.drain()
        wait_clock.add_sem_waits(
            drain_inst.ins, ScopedClock({None: tick_clock.global_clock})
        )
        si = drain_inst.ins.sync_info
        if si is not None and len(si.on_wait) > 1:
            waits = list(si.on_wait)
            si.on_wait = waits[:1]
            for w in waits[1:]:
                n = nc.sync.nop()
                n.ins.sync_info = bass_rust.SyncInfo(on_wait=[w], on_update=[])
        nc.all_engine_barrier()
        assert self.sems is not None
        popped = nc._tile_sem_poison_stack.pop()
        assert popped is self._sem_poison
        nc.clear_and_free_semaphores(list(self.sems.allocated().values()))
        nc.all_engine_barrier()


# --------------------------------------------------------------------------
# host-side sharding / packing
# --------------------------------------------------------------------------

def _shard_and_pack(entity_indices):
    Nn = entity_indices.shape[0]
    starts = np.searchsorted(entity_indices, np.arange(E + 1))
    ideal = (np.arange(1, N_CORES) * Nn) // N_CORES
    ent_bnd = [0]
    for t in ideal:
        s = int(np.searchsorted(starts, t))
        if s > 0 and abs(int(starts[s - 1]) - int(t)) < abs(int(starts[s]) - int(t)):
            s -= 1
        ent_bnd.append(s)
    ent_bnd.append(E)

    cores = []
    for c in range(N_CORES):
        e_lo, e_hi = ent_bnd[c], ent_bnd[c + 1]
        segs = np.arange(e_lo, e_hi)
        sizes = (starts[e_lo + 1 : e_hi + 1] - starts[e_lo:e_hi]).astype(np.int64)
        n_edges = int(sizes.sum())
        assert e_hi - e_lo <= E_PAD and n_edges <= NL
        order = np.argsort(-sizes, kind="stable")
        blk_edges = [0] * NBLK
        blk_nseg = [0] * NBLK
        blk_segs = [[] for _ in range(NBLK)]
        for idx in order:
            sz = int(sizes[idx])
            best = -1
            for b in sorted(range(NBLK), key=lambda b: blk_edges[b]):
                if blk_nseg[b] < SPB and blk_edges[b] + sz <= EPB:
                    best = b
                    break
            assert best >= 0, "block packing overflow"
            blk_segs[best].append(int(segs[idx]))
            blk_edges[best] += sz
            blk_nseg[best] += 1
        perm = np.full(NL, -1, dtype=np.int64)
        seg_local = np.full(NL, PAD_SEG, dtype=np.float32)
        row2seg = np.full(E_PAD, -1, dtype=np.int64)
        inv_cnt = np.zeros(E_PAD, dtype=np.float32)
        for b in range(NBLK):
            pos = b * EPB
            for j, s in enumerate(blk_segs[b]):
                row = b * SPB + j
                row2seg[row] = s
                n = int(starts[s + 1] - starts[s])
                if n > 0:
                    inv_cnt[row] = 1.0 / n
                perm[pos : pos + n] = np.arange(starts[s], starts[s + 1])
                seg_local[pos : pos + n] = float(j)
                pos += n
        cores.append(dict(perm=perm, seg_local=seg_local, row2seg=row2seg,
                          inv_cnt=inv_cnt))
    return cores


# --------------------------------------------------------------------------
# device kernel
# --------------------------------------------------------------------------

def _build_nc():
    nc = bass.Bass("TRN2", target_bir_lowering=False, debug=False,
                   num_devices=N_CORES)

    f32, bf, i32 = dt.float32, dt.bfloat16, dt.int32
    # pass-A edge features (freed mid-block): [ent0:512 | rel0:512]
    enra_d = nc.dram_tensor("enra", [NL, EDA], bf, kind="ExternalInput")
    # pass-B tails (small, deep-buffered): [ent512:768 | rel512:768]
    enrb_d = nc.dram_tensor("enrb", [NL, EDB], bf, kind="ExternalInput")
    cnta_d = nc.dram_tensor("cnta", [NL, 512], bf, kind="ExternalInput")
    cntb_d = nc.dram_tensor("cntb", [NL, CWB], bf, kind="ExternalInput")
    sp_d = nc.dram_tensor("sp", [NL, 2], f32, kind="ExternalInput")
    icnt_d = nc.dram_tensor("inv_cnt", [E_PAD], f32, kind="ExternalInput")
    wenra_d = nc.dram_tensor("wenra", [128, EDA], bf, kind="ExternalInput")
    wenrb_d = nc.dram_tensor("wenrb", [128, EDB], bf, kind="ExternalInput")
    wtr_d = nc.dram_tensor("wtr", [5 * 128, 12 * OH], bf, kind="ExternalInput")
    wte_d = nc.dram_tensor("wte", [5 * 128, 6 * OH], bf, kind="ExternalInput")
    orel_d = nc.dram_tensor("orel", [5 * NBLK * 128, OH], bf,
                            kind="ExternalOutput")
    oent_d = nc.dram_tensor("oent", [5 * NBLK * 128, OH], bf,
                            kind="ExternalOutput")

    with _TileContextSplitDrain(nc) as tc, ExitStack() as es:
        const = es.enter_context(tc.tile_pool(name="const", bufs=1))
        accp = es.enter_context(tc.tile_pool(name="accp", bufs=1))

        # ---- constants ----
        iota_seg = const.tile([128, 128], bf)
        ident = const.tile([128, 128], bf)
        with tc.tile_pool(name="setup", bufs=1) as setup:
            iota_i = setup.tile([128, 128], i32)
            nc.gpsimd.iota(iota_i[:], pattern=[[1, 128]], base=0,
                           channel_multiplier=0)
            nc.vector.tensor_copy(iota_seg[:], iota_i[:])
            iota_ci = setup.tile([128, 1], i32)
            nc.gpsimd.iota(iota_ci[:], pattern=[[0, 1]], base=0,
                           channel_multiplier=1)
            iota_col = setup.tile([128, 1], f32)
            nc.vector.tensor_copy(iota_col[:], iota_ci[:])
            nc.vector.tensor_scalar(out=ident[:], in0=iota_seg[:],
                                    scalar1=iota_col[:],
                                    scalar2=None, op0=Alu.is_equal)

        wenra = const.tile([128, EDA], bf)
        nc.sync.dma_start(wenra[:], wenra_d.ap())
        wenrb = const.tile([128, EDB], bf)
        nc.sync.dma_start(wenrb[:], wenrb_d.ap())
        icnt_sb = const.tile([128, NBLK], f32)
        nc.sync.dma_start(
            icnt_sb[:], icnt_d.ap().rearrange("(b p) -> p b", p=128)
        )
        invd_sb = accp.tile([128, NBLK], f32)

        # resident transposed aggregates, one tile per (feat-chunk, block):
        # t 0-5 = relation, 6-11 = count emb  -> relcat (K=12 chunks)
        # t 0-5 of entT = entity              -> ent (K=6 chunks)
        relcatT = [[accp.tile([128, 128], bf, name=f"relcatT{t}_{b}",
                              tag=f"relcatT{t}_{b}") for b in range(NBLK)]
                   for t in range(12)]
        entT = [[accp.tile([128, 128], bf, name=f"entT{t}_{b}",
                           tag=f"entT{t}_{b}") for b in range(NBLK)]
                for t in range(6)]

        # ---- merged aggregation + projection (Tile interleaves by deps) ----
        HE = CH // 2  # 5 edges per partition per half-block
        with tc.tile_pool(name="edges", bufs=2) as edges, \
             tc.tile_pool(name="chunkp", bufs=2) as chunkp, \
             tc.tile_pool(name="evac", bufs=2) as evac, \
             tc.tile_pool(name="wpool", bufs=2) as wpool, \
             tc.tile_pool(name="outp", bufs=2) as outp, \
             tc.tile_pool(name="psagg", bufs=1, space="PSUM") as psagg, \
             tc.tile_pool(name="pp", bufs=2, space="PSUM") as pp:
            def emit_transposes(items):
                # items: list of (src_slice, dst_tile); alternate evac engine
                for i, (src, dst) in enumerate(items):
                    pt = pp.tile([128, 128], bf, tag="pt")
                    nc.tensor.transpose(pt[:], src, ident[:])
                    if i % 2 == 0:
                        nc.scalar.activation(dst[:], pt[:], Act.Copy)
                    else:
                        nc.vector.tensor_copy(dst[:], pt[:])

            pend_tr = []   # deferred transposes of the previous block
            for b in range(NBLK):
                halves = []
                for hb in range(2):
                    r0 = b * EPB + hb * (EPB // 2)
                    r1 = r0 + EPB // 2
                    enra = edges.tile([128, HE * EDA], bf, tag="enra")
                    nc.sync.dma_start(
                        enra[:],
                        enra_d.ap()[r0:r1, :].rearrange("(p j) d -> p j d", j=HE),
                    )
                    cnta = edges.tile([128, HE * 512], bf, tag="cnta")
                    nc.sync.dma_start(
                        cnta[:],
                        cnta_d.ap()[r0:r1, :].rearrange("(p j) d -> p j d", j=HE),
                    )
                    enrb = edges.tile([128, HE * EDB], bf, tag="enrb", bufs=3)
                    nc.sync.dma_start(
                        enrb[:],
                        enrb_d.ap()[r0:r1, :].rearrange("(p j) d -> p j d", j=HE),
                    )
                    cntb = edges.tile([128, HE * CWB], bf, tag="cntb", bufs=3)
                    nc.sync.dma_start(
                        cntb[:],
                        cntb_d.ap()[r0:r1, :].rearrange("(p j) d -> p j d", j=HE),
                    )
                    sph = edges.tile([128, HE * 2], f32, tag="sph", bufs=3)
                    nc.sync.dma_start(
                        sph[:],
                        sp_d.ap()[r0:r1, :].rearrange("(p j) c -> p j c", j=HE),
                    )
                    halves.append((enra, enrb, cnta, cntb, sph))

                # score + one-hot build for all 10 chunks (persist across
                # the two aggregation passes)
                ohs, ohxs = [], []
                for j in range(CH):
                    enra, enrb, cnta, cntb, sph = halves[j // HE]
                    jj = j % HE
                    slc = sph[:, 2 * jj : 2 * jj + 1]
                    prc = sph[:, 2 * jj + 1 : 2 * jj + 2]
                    scra = chunkp.tile([128, EDA], bf, tag="scra")
                    saA = chunkp.tile([128, 1], f32, tag=f"saA{j}")
                    nc.vector.scalar_tensor_tensor(
                        out=scra[:], in0=enra[:, jj * EDA : (jj + 1) * EDA],
                        scalar=1.0, in1=wenra[:],
                        op0=Alu.mult, op1=Alu.mult, accum_out=saA[:])
                    scrb = chunkp.tile([128, EDB], bf, tag="scrb")
                    saB = chunkp.tile([128, 1], f32, tag=f"saB{j}")
                    nc.vector.scalar_tensor_tensor(
                        out=scrb[:], in0=enrb[:, jj * EDB : (jj + 1) * EDB],
                        scalar=1.0, in1=wenrb[:],
                        op0=Alu.mult, op1=Alu.mult, accum_out=saB[:])
                    sa = chunkp.tile([128, 1], f32, tag=f"sa{j}")
                    nc.vector.tensor_scalar(out=sa[:], in0=saA[:],
                                            scalar1=saB[:], scalar2=None,
                                            op0=Alu.add)
                    ex_ = chunkp.tile([128, 1], f32, tag=f"ex{j}")
                    nc.scalar.activation(ex_[:], sa[:], Act.Exp, bias=prc)
                    oh = chunkp.tile([128, 128], bf, tag=f"oh{j}")
                    nc.vector.tensor_scalar(out=oh[:], in0=iota_seg[:],
                                            scalar1=slc,
                                            scalar2=None, op0=Alu.is_equal)
                    ohx = chunkp.tile([128, 128], bf, tag=f"ohx{j}")
                    nc.vector.tensor_scalar(out=ohx[:], in0=iota_seg[:],
                                            scalar1=slc,
                                            scalar2=ex_[:],
                                            op0=Alu.is_equal, op1=Alu.mult)
                    ohs.append(oh)
                    ohxs.append(ohx)

                relsb = evac.tile([128, D], bf, tag="relsb")
                cntsb = evac.tile([128, D], bf, tag="cntsb")
                entsb = evac.tile([128, D], bf, tag="entsb")

                # pass A: feature cols 0:512 (cntA col 0 is the ones column,
                # so psA_cnt[:,0] accumulates the softmax denominator)
                psA_rel = psagg.tile([128, 512], f32, tag="ps_rel")
                psA_cnt = psagg.tile([128, 512], f32, tag="ps_cnt")
                psA_ent = psagg.tile([128, 512], f32, tag="ps_ent")
                for j in range(CH):
                    enra, enrb, cnta, cntb, sph = halves[j // HE]
                    jj = j % HE
                    ejA = enra[:, jj * EDA : jj * EDA + 512]
                    rjA = enra[:, jj * EDA + 512 : jj * EDA + 1024]
                    cjA = cnta[:, jj * 512 : (jj + 1) * 512]
                    st, sp_ = (j == 0), (j == CH - 1)
                    nc.tensor.matmul(psA_rel[:], ohxs[j][:], rjA,
                                     start=st, stop=sp_)
                    nc.tensor.matmul(psA_cnt[:], ohxs[j][:], cjA,
                                     start=st, stop=sp_)
                    nc.tensor.matmul(psA_ent[:], ohs[j][:], ejA,
                                     start=st, stop=sp_)

                # first half of the previous block's transposes fills the
                # pass-A -> pass-B evac latency on the tensor queue
                emit_transposes(pend_tr[:9])

                dmx = chunkp.tile([128, 1], f32, tag="dmx")
                nc.vector.tensor_scalar(out=dmx[:], in0=psA_cnt[:, 0:1],
                                        scalar1=1e-30, scalar2=None, op0=Alu.max)
                nc.vector.reciprocal(invd_sb[:, b : b + 1], dmx[:])
                nc.scalar.activation(relsb[:, 0:512], psA_rel[:], Act.Copy,
                                     scale=invd_sb[:, b : b + 1])
                nc.scalar.activation(cntsb[:, 0:511], psA_cnt[:, 1:512],
                                     Act.Copy, scale=invd_sb[:, b : b + 1])
                nc.scalar.activation(entsb[:, 0:512], psA_ent[:], Act.Copy,
                                     scale=icnt_sb[:, b : b + 1])

                # pass B: feature cols 512:768 (+ count tail)
                psB_rel = psagg.tile([128, 512], f32, tag="ps_rel")
                psB_cnt = psagg.tile([128, 512], f32, tag="ps_cnt")
                psB_ent = psagg.tile([128, 512], f32, tag="ps_ent")
                for j in range(CH):
                    enra, enrb, cnta, cntb, sph = halves[j // HE]
                    jj = j % HE
                    ejB = enrb[:, jj * EDB : jj * EDB + 256]
                    rjB = enrb[:, jj * EDB + 256 : (jj + 1) * EDB]
                    cjB = cntb[:, jj * CWB : (jj + 1) * CWB]
                    st, sp_ = (j == 0), (j == CH - 1)
                    nc.tensor.matmul(psB_rel[:, 0:256], ohxs[j][:], rjB,
                                     start=st, stop=sp_)
                    nc.tensor.matmul(psB_cnt[:, 0:CWB], ohxs[j][:], cjB,
                                     start=st, stop=sp_)
                    nc.tensor.matmul(psB_ent[:, 0:256], ohs[j][:], ejB,
                                     start=st, stop=sp_)

                # second half of the previous block's transposes fills the
                # block-boundary evac latency
                emit_transposes(pend_tr[9:])

                nc.scalar.activation(relsb[:, 512:D], psB_rel[:, 0:256],
                                     Act.Copy, scale=invd_sb[:, b : b + 1])
                nc.scalar.activation(cntsb[:, 511:D], psB_cnt[:, 0:257],
                                     Act.Copy, scale=invd_sb[:, b : b + 1])
                nc.scalar.activation(entsb[:, 512:D], psB_ent[:, 0:256],
                                     Act.Copy, scale=icnt_sb[:, b : b + 1])

                pend_tr = []
                for t in range(6):
                    pend_tr.append((relsb[:, t * 128 : (t + 1) * 128],
                                    relcatT[t][b]))
                    pend_tr.append((cntsb[:, t * 128 : (t + 1) * 128],
                                    relcatT[6 + t][b]))
                    pend_tr.append((entsb[:, t * 128 : (t + 1) * 128],
                                    entT[t][b]))
            emit_transposes(pend_tr)

            # ---- projections (interleave with later aggregation blocks) ----
            for (Tt, wt_d, o_d, KC) in (
                (relcatT, wtr_d, orel_d, 12),
                (entT, wte_d, oent_d, 6),
            ):
                for h in range(5):
                    wt = wpool.tile([128, KC * OH], bf, tag="wt")
                    nc.gpsimd.dma_start(
                        wt[:], wt_d.ap()[h * 128 : (h + 1) * 128, :])
                    for sblk in range(NBLK):
                        stage = outp.tile([128, OH], bf, tag="stage", bufs=4)
                        for oc5 in range(OH // 512):
                            pso = pp.tile([128, 512], f32, tag="pso", bufs=3)
                            for k in range(KC):
                                nc.tensor.matmul(
                                    pso[:],
                                    Tt[k][sblk][:],
                                    wt[:, k * OH + oc5 * 512 : k * OH + oc5 * 512 + 512],
                                    start=(k == 0), stop=(k == KC - 1))
                            if oc5 % 2 == 0:
                                nc.vector.tensor_copy(
                                    stage[:, oc5 * 512 : (oc5 + 1) * 512], pso[:])
                            else:
                                nc.scalar.activation(
                                    stage[:, oc5 * 512 : (oc5 + 1) * 512], pso[:],
                                    Act.Copy)
                        nc.scalar.dma_start(
                            o_d.ap()[(h * NBLK + sblk) * 128 :
                                     (h * NBLK + sblk + 1) * 128, :],
                            stage[:],
                        )
    return nc


_NC_CACHE = None


def _get_nc():
    global _NC_CACHE
    if _NC_CACHE is None:
        _NC_CACHE = _build_nc()
    return _NC_CACHE


# --------------------------------------------------------------------------
# entry point
# --------------------------------------------------------------------------

def kernel(prompt_embs, entity_embs, neighbor_embs, relation_embs,
           count_table, scorer_W, scorer_b, rel_W, rel_b, ent_W, ent_b,
           counts, prompt_indices, entity_indices):
    from concourse.bass_utils import run_bass_kernel_spmd

    prompt_embs = np.asarray(prompt_embs, dtype=np.float32)
    entity_embs = np.asarray(entity_embs, dtype=np.float32)
    neighbor_embs = np.asarray(neighbor_embs, dtype=np.float32)
    relation_embs = np.asarray(relation_embs, dtype=np.float32)
    count_table = np.asarray(count_table, dtype=np.float32)
    scorer_W = np.asarray(scorer_W, dtype=np.float32)
    scorer_b = np.asarray(scorer_b, dtype=np.float32)
    rel_W = np.asarray(rel_W, dtype=np.float32)
    rel_b = np.asarray(rel_b, dtype=np.float32)
    ent_W = np.asarray(ent_W, dtype=np.float32)
    ent_b = np.asarray(ent_b, dtype=np.float32)
    counts = np.asarray(counts)
    prompt_indices = np.asarray(prompt_indices)
    entity_indices = np.asarray(entity_indices)

    cores = _shard_and_pack(entity_indices)

    # replicated (weight-derived) host prep
    w = scorer_W[0]
    pscore = (prompt_embs * w[None, :D]).sum(1) + scorer_b[0]     # fold bias
    cscore = (count_table * w[None, 4 * D :]).sum(1)
    w_ent, w_nbr, w_rel = w[D : 2 * D], w[2 * D : 3 * D], w[3 * D : 4 * D]
    nscore = (neighbor_embs * w_nbr[None, :]).sum(1)   # fold neighbor dot
    wenra = np.broadcast_to(
        np.concatenate([w_ent[0:512], w_rel[0:512]]).astype(BF16),
        (128, EDA)).copy()
    wenrb = np.broadcast_to(
        np.concatenate([w_ent[512:D], w_rel[512:D]]).astype(BF16),
        (128, EDB)).copy()

    # pass-A edge features [ent0:512 | rel0:512], pass-B tails
    enra_full = np.empty((N, EDA), BF16)
    enra_full[:, 0:512] = entity_embs[:, 0:512].astype(BF16)
    enra_full[:, 512:] = relation_embs[:, 0:512].astype(BF16)
    enrb_full = np.empty((N, EDB), BF16)
    enrb_full[:, 0:256] = entity_embs[:, 512:D].astype(BF16)
    enrb_full[:, 256:512] = relation_embs[:, 512:D].astype(BF16)
    ct_bf = count_table.astype(BF16)
    # per-edge prescore = prompt + count + neighbor scores (+ bias)
    pres_full = (pscore[prompt_indices] + cscore[counts]
                 + nscore).astype(np.float32)

    # pre-tiled projection weights: [h*128+p, k*OH+c] = W[h*OH+c, k*128+p]
    def tile_w(W, KC):
        WT = np.ascontiguousarray(W.T).astype(BF16)          # [K*128, OUT]
        return np.ascontiguousarray(
            WT.reshape(KC, 128, 5, OH).transpose(2, 1, 0, 3)
        ).reshape(5 * 128, KC * OH)

    wtr = tile_w(rel_W, 12)
    wte = tile_w(ent_W, 6)

    in_maps = []
    for core in cores:
        perm = core["perm"]
        valid = perm >= 0
        src = np.where(valid, perm, 0)

        enra = enra_full[src]
        enra[~valid] = 0
        enrb = enrb_full[src]
        enrb[~valid] = 0
        ce = ct_bf[counts[src]]
        ce[~valid] = 0
        cnta = np.zeros((NL, 512), BF16)
        cnta[:, 0] = 1.0         # ones col -> softmax denominator (pass A)
        cnta[~valid, 0] = 0
        cnta[:, 1:512] = ce[:, 0:511]
        cntb = np.zeros((NL, CWB), BF16)
        cntb[:, 0:257] = ce[:, 511:D]
        sp = np.zeros((NL, 2), np.float32)
        sp[:, 0] = core["seg_local"]
        sp[:, 1] = pres_full[src]
        sp[~valid, 1] = 0.0

        in_maps.append(dict(
            enra=np.ascontiguousarray(enra), enrb=np.ascontiguousarray(enrb),
            cnta=cnta, cntb=cntb,
            sp=sp, inv_cnt=core["inv_cnt"], wenra=wenra, wenrb=wenrb,
            wtr=wtr, wte=wte,
        ))

    nc = _get_nc()
    res = run_bass_kernel_spmd(nc, in_maps, list(range(N_CORES)))

    rel_out = np.zeros((E, OUT), np.float32)
    ent_out = np.zeros((E, OUT), np.float32)
    for c, core in enumerate(cores):
        rows = core["row2seg"]
        mask = rows >= 0
        # output DRAM layout [5h x 10blk x 128p, 1024c] -> [1280, 5120]
        orel = np.asarray(res.results[c]["orel"], dtype=np.float32)
        oent = np.asarray(res.results[c]["oent"], dtype=np.float32)
        orel = orel.reshape(5, NBLK * 128, OH).transpose(1, 0, 2).reshape(E_PAD, OUT)
        oent = oent.reshape(5, NBLK * 128, OH).transpose(1, 0, 2).reshape(E_PAD, OUT)
        rel_out[rows[mask]] = orel[mask]
        ent_out[rows[mask]] = oent[mask]
    rel_out += rel_b[None, :]
    ent_out += ent_b[None, :]
    return rel_out, ent_out


# revision 27
# speedup vs baseline: 1.7200x; 1.0140x over previous
"""EntityEncoder (gnn_message_passing) Trainium2 kernel — 8-core SPMD.

Strategy: edges are pre-partitioned on the host into 8 contiguous,
entity-aligned, edge-balanced shards (entity_indices is sorted, so each
entity's edges land wholly on one core — no cross-core collectives).
Within a core, segments are LPT-packed into 10 blocks of <=128 segments /
<=1280 edges.  All HBM traffic is bf16.  The host folds the prompt-score,
count-score and scorer bias into a per-edge prescore, gathers per-edge
count embeddings (with an appended ones column that yields the softmax
denominator for free), and pre-tiles the projection weights.  On device:
one fused 2304-col dot per 128-edge chunk (vector), exp on scalar,
one-hot segment matmuls on tensor for the three segment reductions,
PE transposes of the [seg,feat] aggregates, then bf16 output projections.
Projection bias and the final row scatter are applied on the host.
"""
import sys
import numpy as np
import ml_dtypes

for _p in ("/root/.axon_site", "/root/.axon_site/_ro/trn_rl_repo",
           "/root/.axon_site/_ro/pypackages"):
    if _p not in sys.path:
        sys.path.append(_p)

import bass_rust
import concourse.bass as bass
import concourse.mybir as mybir
import concourse.tile as tile
from concourse.vector_clock import ScopedClock
from contextlib import ExitStack

BF16 = ml_dtypes.bfloat16
dt = mybir.dt
Alu = mybir.AluOpType
Act = mybir.ActivationFunctionType

# problem shape (hardcoded per contest contract)
N_CORES = 8
N = 100_000
P = 64
E = 10_000
D = 768
C = 1000
OUT = 5120
# per-core packing
NBLK = 10
SPB = 128                # segs per block
CH = 10                  # chunks (of 128 edges) per block
EPB = CH * 128           # edges per block = 1280
NL = NBLK * EPB          # 12800 edge slots per core
E_PAD = NBLK * SPB       # 1280 seg slots per core
OH = OUT // 5            # 1024-wide output slab
PAD_SEG = 999.0
EDA = 1024               # pass-A score cols: ent0(512) + rel0(512)
EDB = 512                # pass-B score cols: ent1(256) + rel1(256)
CWB = 264                # pass-B count cols: ce[511:768] + 7 pad
BGA = 1536               # pass-A tensor: e0 | r0 | [ones|ce0:511]
TLB = EDB + CWB          # pass-B tensor: e1 | r1 | ceB = 776


class _TileContextSplitDrain(tile.TileContext):
    """This container's walrus accepts only ONE sync wait per instruction
    ("Too many sync wait commands" in setupSyncWait). Split every extra wait
    onto a standalone same-engine NoOp placed immediately before the
    instruction — identical semantics, one wait per instruction."""

    def _lower_ordered_insts(self, ordered):
        for insts in ordered.values():
            if not any(
                i.sync_info is not None and len(i.sync_info.on_wait) > 1
                for i in insts
            ):
                continue
            new = []
            for inst in insts:
                si = inst.sync_info
                if si is not None and len(si.on_wait) > 1:
                    waits = list(si.on_wait)
                    for w in waits[:-1]:
                        nop = bass_rust.InstNoOp(
                            name=self.nc.get_next_instruction_name(),
                            ins=[], outs=[])
                        nop.engine = inst.engine
                        nop.sync_info = bass_rust.SyncInfo(
                            on_wait=[w], on_update=[])
                        new.append(nop)
                    si.on_wait = waits[-1:]
                new.append(inst)
            insts[:] = new
        return super()._lower_ordered_insts(ordered)

    def _drain_and_barrier(self, tick_clock, wait_clock):
        nc = self.nc
        drain_inst = nc.sync.drain()
        wait_clock.add_sem_waits(
            drain_inst.ins, ScopedClock({None: tick_clock.global_clock})
        )
        si = drain_inst.ins.sync_info
        if si is not None and len(si.on_wait) > 1:
            waits = list(si.on_wait)
            si.on_wait = waits[:1]
            for w in waits[1:]:
                n = nc.sync.nop()
                n.ins.sync_info = bass_rust.SyncInfo(on_wait=[w], on_update=[])
        nc.all_engine_barrier()
        assert self.sems is not None
        popped = nc._tile_sem_poison_stack.pop()
        assert popped is self._sem_poison
        nc.clear_and_free_semaphores(list(self.sems.allocated().values()))
        nc.all_engine_barrier()


# --------------------------------------------------------------------------
# host-side sharding / packing
# --------------------------------------------------------------------------

def _shard_and_pack(entity_indices):
    Nn = entity_indices.shape[0]
    starts = np.searchsorted(entity_indices, np.arange(E + 1))
    ideal = (np.arange(1, N_CORES) * Nn) // N_CORES
    ent_bnd = [0]
    for t in ideal:
        s = int(np.searchsorted(starts, t))
        if s > 0 and abs(int(starts[s - 1]) - int(t)) < abs(int(starts[s]) - int(t)):
            s -= 1
        ent_bnd.append(s)
    ent_bnd.append(E)

    cores = []
    for c in range(N_CORES):
        e_lo, e_hi = ent_bnd[c], ent_bnd[c + 1]
        segs = np.arange(e_lo, e_hi)
        sizes = (starts[e_lo + 1 : e_hi + 1] - starts[e_lo:e_hi]).astype(np.int64)
        n_edges = int(sizes.sum())
        assert e_hi - e_lo <= E_PAD and n_edges <= NL
        order = np.argsort(-sizes, kind="stable")
        blk_edges = [0] * NBLK
        blk_nseg = [0] * NBLK
        blk_segs = [[] for _ in range(NBLK)]
        for idx in order:
            sz = int(sizes[idx])
            best = -1
            for b in sorted(range(NBLK), key=lambda b: blk_edges[b]):
                if blk_nseg[b] < SPB and blk_edges[b] + sz <= EPB:
                    best = b
                    break
            assert best >= 0, "block packing overflow"
            blk_segs[best].append(int(segs[idx]))
            blk_edges[best] += sz
            blk_nseg[best] += 1
        perm = np.full(NL, -1, dtype=np.int64)
        seg_local = np.full(NL, PAD_SEG, dtype=np.float32)
        row2seg = np.full(E_PAD, -1, dtype=np.int64)
        inv_cnt = np.zeros(E_PAD, dtype=np.float32)
        for b in range(NBLK):
            pos = b * EPB
            for j, s in enumerate(blk_segs[b]):
                row = b * SPB + j
                row2seg[row] = s
                n = int(starts[s + 1] - starts[s])
                if n > 0:
                    inv_cnt[row] = 1.0 / n
                perm[pos : pos + n] = np.arange(starts[s], starts[s + 1])
                seg_local[pos : pos + n] = float(j)
                pos += n
        cores.append(dict(perm=perm, seg_local=seg_local, row2seg=row2seg,
                          inv_cnt=inv_cnt))
    return cores


# --------------------------------------------------------------------------
# device kernel
# --------------------------------------------------------------------------

def _build_nc():
    nc = bass.Bass("TRN2", target_bir_lowering=False, debug=False,
                   num_devices=N_CORES)

    f32, bf, i32 = dt.float32, dt.bfloat16, dt.int32
    # pass-A edge features (freed mid-block): [e0 512 | r0 512 | ones+ce0 512]
    biga_d = nc.dram_tensor("biga", [NL, BGA], bf, kind="ExternalInput")
    # pass-B tails (small, deep-buffered): [e1 256 | r1 256 | ceB 264]
    tailb_d = nc.dram_tensor("tailb", [NL, TLB], bf, kind="ExternalInput")
    sp_d = nc.dram_tensor("sp", [NL, 2], f32, kind="ExternalInput")
    icnt_d = nc.dram_tensor("inv_cnt", [E_PAD], f32, kind="ExternalInput")
    wenra_d = nc.dram_tensor("wenra", [128, EDA], bf, kind="ExternalInput")
    wenrb_d = nc.dram_tensor("wenrb", [128, EDB], bf, kind="ExternalInput")
    wtr_d = nc.dram_tensor("wtr", [5 * 128, 12 * OH], bf, kind="ExternalInput")
    wte_d = nc.dram_tensor("wte", [5 * 128, 6 * OH], bf, kind="ExternalInput")
    orel_d = nc.dram_tensor("orel", [5 * NBLK * 128, OH], bf,
                            kind="ExternalOutput")
    oent_d = nc.dram_tensor("oent", [5 * NBLK * 128, OH], bf,
                            kind="ExternalOutput")

    with _TileContextSplitDrain(nc) as tc, ExitStack() as es:
        const = es.enter_context(tc.tile_pool(name="const", bufs=1))
        accp = es.enter_context(tc.tile_pool(name="accp", bufs=1))

        # ---- constants ----
        iota_seg = const.tile([128, 128], bf)
        ident = const.tile([128, 128], bf)
        with tc.tile_pool(name="setup", bufs=1) as setup:
            iota_i = setup.tile([128, 128], i32)
            nc.gpsimd.iota(iota_i[:], pattern=[[1, 128]], base=0,
                           channel_multiplier=0)
            nc.vector.tensor_copy(iota_seg[:], iota_i[:])
            iota_ci = setup.tile([128, 1], i32)
            nc.gpsimd.iota(iota_ci[:], pattern=[[0, 1]], base=0,
                           channel_multiplier=1)
            iota_col = setup.tile([128, 1], f32)
            nc.vector.tensor_copy(iota_col[:], iota_ci[:])
            nc.vector.tensor_scalar(out=ident[:], in0=iota_seg[:],
                                    scalar1=iota_col[:],
                                    scalar2=None, op0=Alu.is_equal)

        wenra = const.tile([128, EDA], bf)
        nc.sync.dma_start(wenra[:], wenra_d.ap())
        wenrb = const.tile([128, EDB], bf)
        nc.sync.dma_start(wenrb[:], wenrb_d.ap())
        icnt_sb = const.tile([128, NBLK], f32)
        nc.sync.dma_start(
            icnt_sb[:], icnt_d.ap().rearrange("(b p) -> p b", p=128)
        )
        invd_sb = accp.tile([128, NBLK], f32)

        # resident transposed aggregates, one tile per (feat-chunk, block):
        # t 0-5 = relation, 6-11 = count emb  -> relcat (K=12 chunks)
        # t 0-5 of entT = entity              -> ent (K=6 chunks)
        relcatT = [[accp.tile([128, 128], bf, name=f"relcatT{t}_{b}",
                              tag=f"relcatT{t}_{b}") for b in range(NBLK)]
                   for t in range(12)]
        entT = [[accp.tile([128, 128], bf, name=f"entT{t}_{b}",
                           tag=f"entT{t}_{b}") for b in range(NBLK)]
                for t in range(6)]

        # ---- merged aggregation + projection (Tile interleaves by deps) ----
        HE = CH // 2  # 5 edges per partition per half-block
        with tc.tile_pool(name="edges", bufs=2) as edges, \
             tc.tile_pool(name="chunkp", bufs=2) as chunkp, \
             tc.tile_pool(name="evac", bufs=2) as evac, \
             tc.tile_pool(name="wpool", bufs=2) as wpool, \
             tc.tile_pool(name="outp", bufs=2) as outp, \
             tc.tile_pool(name="psagg", bufs=1, space="PSUM") as psagg, \
             tc.tile_pool(name="pp", bufs=2, space="PSUM") as pp:
            def emit_transposes(items):
                # items: list of (src_slice, dst_tile); alternate evac engine
                for i, (src, dst) in enumerate(items):
                    pt = pp.tile([128, 128], bf, tag="pt")
                    nc.tensor.transpose(pt[:], src, ident[:])
                    if i % 2 == 0:
                        nc.scalar.activation(dst[:], pt[:], Act.Copy)
                    else:
                        nc.vector.tensor_copy(dst[:], pt[:])

            pend_tr = []   # deferred transposes of the previous block
            for b in range(NBLK):
                halves = []
                for hb in range(2):
                    r0 = b * EPB + hb * (EPB // 2)
                    r1 = r0 + EPB // 2
                    biga = edges.tile([128, HE * BGA], bf, tag="biga")
                    nc.sync.dma_start(
                        biga[:],
                        biga_d.ap()[r0:r1, :].rearrange("(p j) d -> p j d", j=HE),
                    )
                    tailb = edges.tile([128, HE * TLB], bf, tag="tailb", bufs=3)
                    nc.sync.dma_start(
                        tailb[:],
                        tailb_d.ap()[r0:r1, :].rearrange("(p j) d -> p j d", j=HE),
                    )
                    sph = edges.tile([128, HE * 2], f32, tag="sph", bufs=3)
                    nc.sync.dma_start(
                        sph[:],
                        sp_d.ap()[r0:r1, :].rearrange("(p j) c -> p j c", j=HE),
                    )
                    halves.append((biga, tailb, sph))
                    if b == 0 and hb == 0:
                        # gate the (gpsimd-queued) weight stream behind the
                        # first edge load so startup DMA bandwidth goes to
                        # block 0
                        gate = chunkp.tile([1, 1], bf, tag="gate", bufs=1)
                        nc.gpsimd.tensor_copy(gate[:], biga[0:1, 0:1])

                # score + one-hot build for all 10 chunks (persist across
                # the two aggregation passes)
                ohs, ohxs = [], []
                for j in range(CH):
                    biga, tailb, sph = halves[j // HE]
                    jj = j % HE
                    slc = sph[:, 2 * jj : 2 * jj + 1]
                    prc = sph[:, 2 * jj + 1 : 2 * jj + 2]
                    scra = chunkp.tile([128, EDA], bf, tag="scra")
                    saA = chunkp.tile([128, 1], f32, tag=f"saA{j}")
                    nc.vector.scalar_tensor_tensor(
                        out=scra[:], in0=biga[:, jj * BGA : jj * BGA + EDA],
                        scalar=1.0, in1=wenra[:],
                        op0=Alu.mult, op1=Alu.mult, accum_out=saA[:])
                    scrb = chunkp.tile([128, EDB], bf, tag="scrb")
                    saB = chunkp.tile([128, 1], f32, tag=f"saB{j}")
                    nc.vector.scalar_tensor_tensor(
                        out=scrb[:], in0=tailb[:, jj * TLB : jj * TLB + EDB],
                        scalar=1.0, in1=wenrb[:],
                        op0=Alu.mult, op1=Alu.mult, accum_out=saB[:])
                    sa = chunkp.tile([128, 1], f32, tag=f"sa{j}")
                    nc.vector.tensor_scalar(out=sa[:], in0=saA[:],
                                            scalar1=saB[:], scalar2=None,
                                            op0=Alu.add)
                    ex_ = chunkp.tile([128, 1], f32, tag=f"ex{j}")
                    nc.scalar.activation(ex_[:], sa[:], Act.Exp, bias=prc)
                    oh = chunkp.tile([128, 128], bf, tag=f"oh{j}")
                    nc.vector.tensor_scalar(out=oh[:], in0=iota_seg[:],
                                            scalar1=slc,
                                            scalar2=None, op0=Alu.is_equal)
                    ohx = chunkp.tile([128, 128], bf, tag=f"ohx{j}")
                    nc.vector.tensor_scalar(out=ohx[:], in0=iota_seg[:],
                                            scalar1=slc,
                                            scalar2=ex_[:],
                                            op0=Alu.is_equal, op1=Alu.mult)
                    ohs.append(oh)
                    ohxs.append(ohx)

                relsb = evac.tile([128, D], bf, tag="relsb")
                cntsb = evac.tile([128, D], bf, tag="cntsb")
                entsb = evac.tile([128, D], bf, tag="entsb")

                # pass A: feature cols 0:512 (cntA col 0 is the ones column,
                # so psA_cnt[:,0] accumulates the softmax denominator)
                psA_rel = psagg.tile([128, 512], f32, tag="ps_rel")
                psA_cnt = psagg.tile([128, 512], f32, tag="ps_cnt")
                psA_ent = psagg.tile([128, 512], f32, tag="ps_ent")
                for j in range(CH):
                    biga, tailb, sph = halves[j // HE]
                    jj = j % HE
                    ejA = biga[:, jj * BGA : jj * BGA + 512]
                    rjA = biga[:, jj * BGA + 512 : jj * BGA + 1024]
                    cjA = biga[:, jj * BGA + 1024 : jj * BGA + 1536]
                    st, sp_ = (j == 0), (j == CH - 1)
                    nc.tensor.matmul(psA_rel[:], ohxs[j][:], rjA,
                                     start=st, stop=sp_)
                    nc.tensor.matmul(psA_cnt[:], ohxs[j][:], cjA,
                                     start=st, stop=sp_)
                    nc.tensor.matmul(psA_ent[:], ohs[j][:], ejA,
                                     start=st, stop=sp_)

                # first half of the previous block's transposes fills the
                # pass-A -> pass-B evac latency on the tensor queue
                emit_transposes(pend_tr[:9])

                dmx = chunkp.tile([128, 1], f32, tag="dmx")
                nc.vector.tensor_scalar(out=dmx[:], in0=psA_cnt[:, 0:1],
                                        scalar1=1e-30, scalar2=None, op0=Alu.max)
                nc.vector.reciprocal(invd_sb[:, b : b + 1], dmx[:])
                nc.scalar.activation(relsb[:, 0:512], psA_rel[:], Act.Copy,
                                     scale=invd_sb[:, b : b + 1])
                nc.scalar.activation(cntsb[:, 0:511], psA_cnt[:, 1:512],
                                     Act.Copy, scale=invd_sb[:, b : b + 1])
                nc.scalar.activation(entsb[:, 0:512], psA_ent[:], Act.Copy,
                                     scale=icnt_sb[:, b : b + 1])

                # pass B: feature cols 512:768 (+ count tail)
                psB_rel = psagg.tile([128, 512], f32, tag="ps_rel")
                psB_cnt = psagg.tile([128, 512], f32, tag="ps_cnt")
                psB_ent = psagg.tile([128, 512], f32, tag="ps_ent")
                for j in range(CH):
                    biga, tailb, sph = halves[j // HE]
                    jj = j % HE
                    ejB = tailb[:, jj * TLB : jj * TLB + 256]
                    rjB = tailb[:, jj * TLB + 256 : jj * TLB + 512]
                    cjB = tailb[:, jj * TLB + 512 : (jj + 1) * TLB]
                    st, sp_ = (j == 0), (j == CH - 1)
                    nc.tensor.matmul(psB_rel[:, 0:256], ohxs[j][:], rjB,
                                     start=st, stop=sp_)
                    nc.tensor.matmul(psB_cnt[:, 0:CWB], ohxs[j][:], cjB,
                                     start=st, stop=sp_)
                    nc.tensor.matmul(psB_ent[:, 0:256], ohs[j][:], ejB,
                                     start=st, stop=sp_)

                # second half of the previous block's transposes fills the
                # block-boundary evac latency
                emit_transposes(pend_tr[9:])

                nc.scalar.activation(relsb[:, 512:D], psB_rel[:, 0:256],
                                     Act.Copy, scale=invd_sb[:, b : b + 1])
                nc.scalar.activation(cntsb[:, 511:D], psB_cnt[:, 0:257],
                                     Act.Copy, scale=invd_sb[:, b : b + 1])
                nc.scalar.activation(entsb[:, 512:D], psB_ent[:, 0:256],
                                     Act.Copy, scale=icnt_sb[:, b : b + 1])

                pend_tr = []
                for t in range(6):
                    pend_tr.append((relsb[:, t * 128 : (t + 1) * 128],
                                    relcatT[t][b]))
                    pend_tr.append((cntsb[:, t * 128 : (t + 1) * 128],
                                    relcatT[6 + t][b]))
                    pend_tr.append((entsb[:, t * 128 : (t + 1) * 128],
                                    entT[t][b]))
            emit_transposes(pend_tr)

            # ---- projections (interleave with later aggregation blocks) ----
            for (Tt, wt_d, o_d, KC) in (
                (relcatT, wtr_d, orel_d, 12),
                (entT, wte_d, oent_d, 6),
            ):
                for h in range(5):
                    wt = wpool.tile([128, KC * OH], bf, tag="wt")
                    nc.gpsimd.dma_start(
                        wt[:], wt_d.ap()[h * 128 : (h + 1) * 128, :])
                    for sblk in range(NBLK):
                        stage = outp.tile([128, OH], bf, tag="stage", bufs=4)
                        for oc5 in range(OH // 512):
                            pso = pp.tile([128, 512], f32, tag="pso", bufs=3)
                            for k in range(KC):
                                nc.tensor.matmul(
                                    pso[:],
                                    Tt[k][sblk][:],
                                    wt[:, k * OH + oc5 * 512 : k * OH + oc5 * 512 + 512],
                                    start=(k == 0), stop=(k == KC - 1))
                            if oc5 % 2 == 0:
                                nc.vector.tensor_copy(
                                    stage[:, oc5 * 512 : (oc5 + 1) * 512], pso[:])
                            else:
                                nc.scalar.activation(
                                    stage[:, oc5 * 512 : (oc5 + 1) * 512], pso[:],
                                    Act.Copy)
                        nc.scalar.dma_start(
                            o_d.ap()[(h * NBLK + sblk) * 128 :
                                     (h * NBLK + sblk + 1) * 128, :],
                            stage[:],
                        )
    return nc


_NC_CACHE = None


def _get_nc():
    global _NC_CACHE
    if _NC_CACHE is None:
        _NC_CACHE = _build_nc()
    return _NC_CACHE


# --------------------------------------------------------------------------
# entry point
# --------------------------------------------------------------------------

def kernel(prompt_embs, entity_embs, neighbor_embs, relation_embs,
           count_table, scorer_W, scorer_b, rel_W, rel_b, ent_W, ent_b,
           counts, prompt_indices, entity_indices):
    from concourse.bass_utils import run_bass_kernel_spmd

    prompt_embs = np.asarray(prompt_embs, dtype=np.float32)
    entity_embs = np.asarray(entity_embs, dtype=np.float32)
    neighbor_embs = np.asarray(neighbor_embs, dtype=np.float32)
    relation_embs = np.asarray(relation_embs, dtype=np.float32)
    count_table = np.asarray(count_table, dtype=np.float32)
    scorer_W = np.asarray(scorer_W, dtype=np.float32)
    scorer_b = np.asarray(scorer_b, dtype=np.float32)
    rel_W = np.asarray(rel_W, dtype=np.float32)
    rel_b = np.asarray(rel_b, dtype=np.float32)
    ent_W = np.asarray(ent_W, dtype=np.float32)
    ent_b = np.asarray(ent_b, dtype=np.float32)
    counts = np.asarray(counts)
    prompt_indices = np.asarray(prompt_indices)
    entity_indices = np.asarray(entity_indices)

    cores = _shard_and_pack(entity_indices)

    # replicated (weight-derived) host prep
    w = scorer_W[0]
    pscore = (prompt_embs * w[None, :D]).sum(1) + scorer_b[0]     # fold bias
    cscore = (count_table * w[None, 4 * D :]).sum(1)
    w_ent, w_nbr, w_rel = w[D : 2 * D], w[2 * D : 3 * D], w[3 * D : 4 * D]
    nscore = (neighbor_embs * w_nbr[None, :]).sum(1)   # fold neighbor dot
    wenra = np.broadcast_to(
        np.concatenate([w_ent[0:512], w_rel[0:512]]).astype(BF16),
        (128, EDA)).copy()
    wenrb = np.broadcast_to(
        np.concatenate([w_ent[512:D], w_rel[512:D]]).astype(BF16),
        (128, EDB)).copy()

    # pass-A edge features [ent0:512 | rel0:512 | ones+ce], pass-B tails
    enra_full = np.empty((N, EDA), BF16)
    enra_full[:, 0:512] = entity_embs[:, 0:512].astype(BF16)
    enra_full[:, 512:] = relation_embs[:, 0:512].astype(BF16)
    enrb_full = np.empty((N, EDB), BF16)
    enrb_full[:, 0:256] = entity_embs[:, 512:D].astype(BF16)
    enrb_full[:, 256:512] = relation_embs[:, 512:D].astype(BF16)
    ct_bf = count_table.astype(BF16)
    # per-edge prescore = prompt + count + neighbor scores (+ bias)
    pres_full = (pscore[prompt_indices] + cscore[counts]
                 + nscore).astype(np.float32)

    # pre-tiled projection weights: [h*128+p, k*OH+c] = W[h*OH+c, k*128+p]
    def tile_w(W, KC):
        WT = np.ascontiguousarray(W.T).astype(BF16)          # [K*128, OUT]
        return np.ascontiguousarray(
            WT.reshape(KC, 128, 5, OH).transpose(2, 1, 0, 3)
        ).reshape(5 * 128, KC * OH)

    wtr = tile_w(rel_W, 12)
    wte = tile_w(ent_W, 6)

    in_maps = []
    for core in cores:
        perm = core["perm"]
        valid = perm >= 0
        src = np.where(valid, perm, 0)

        ce = ct_bf[counts[src]]
        ce[~valid] = 0
        biga = np.zeros((NL, BGA), BF16)
        biga[:, 0:EDA] = enra_full[src]
        biga[~valid, 0:EDA] = 0
        biga[:, 1024] = 1.0      # ones col -> softmax denominator (pass A)
        biga[~valid, 1024] = 0
        biga[:, 1025:1536] = ce[:, 0:511]
        tailb = np.zeros((NL, TLB), BF16)
        tailb[:, 0:512] = enrb_full[src]
        tailb[~valid, 0:512] = 0
        tailb[:, 512 : 512 + 257] = ce[:, 511:D]
        sp = np.zeros((NL, 2), np.float32)
        sp[:, 0] = core["seg_local"]
        sp[:, 1] = pres_full[src]
        sp[~valid, 1] = 0.0

        in_maps.append(dict(
            biga=biga, tailb=tailb,
            sp=sp, inv_cnt=core["inv_cnt"], wenra=wenra, wenrb=wenrb,
            wtr=wtr, wte=wte,
        ))

    nc = _get_nc()
    res = run_bass_kernel_spmd(nc, in_maps, list(range(N_CORES)))

    rel_out = np.zeros((E, OUT), np.float32)
    ent_out = np.zeros((E, OUT), np.float32)
    for c, core in enumerate(cores):
        rows = core["row2seg"]
        mask = rows >= 0
        # output DRAM layout [5h x 10blk x 128p, 1024c] -> [1280, 5120]
        orel = np.asarray(res.results[c]["orel"], dtype=np.float32)
        oent = np.asarray(res.results[c]["oent"], dtype=np.float32)
        orel = orel.reshape(5, NBLK * 128, OH).transpose(1, 0, 2).reshape(E_PAD, OUT)
        oent = oent.reshape(5, NBLK * 128, OH).transpose(1, 0, 2).reshape(E_PAD, OUT)
        rel_out[rows[mask]] = orel[mask]
        ent_out[rows[mask]] = oent[mask]
    rel_out += rel_b[None, :]
    ent_out += ent_b[None, :]
    return rel_out, ent_out
